# revision 39
# baseline (speedup 1.0000x reference)
"""Causal self-attention (B=4, T=2048, C=1024, H=16, D=64) on 8 TRN2 cores.

Sharding: core = 2*b + hg  (b = batch 0..3, hg = head-group 0..1 of 8 heads).
Each core computes its batch's QKV projections for its 8 heads (tensor
parallel over wq/wk/wv rows), RMSNorm+RoPE, causal attention, and a partial
output projection over its head-group's wproj columns. The two partials per
batch are summed on the host.

v7: fp8e4 DoubleRow matmuls with error-feedback for QKV and PV.
QKV runs three accumulating DoubleRow passes -- x8@w8 + dx8@w8 + x8@dw8 --
where dx8/dw8 are the fp8 quantization residuals (prepared on host, w
pre-scaled x64: the q/k scale cancels in RMSNorm, the v scale folds into the
value blend).  This cuts QKV PE time 25% below fp16 at ~0.1% extra error.
Values are stored as v8 + dv8 planes; PV runs two DoubleRow passes per
kt-pair (pt8 x v8, pt8 x dv8), contraction 256, with the ones-row riding in
v8 (zeros in dv8) for softmax denominators.  Probabilities go to fp8
directly from the ACT exp (bias -3 keeps exp inside fp8 range and cancels
in normalization).  Scores stay fp16 (qT/kT via DMA XBAR transpose);
RMSNorm applies AFTER RoPE (rotation commutes with the per-head scale).
Diagonal-strip exps cover both heads via one strided ACT instruction;
triangle masks use [tri]/[0|tri] tables on gpsimd (SBUF-only: TRN2 gpsimd
cannot access PSUM, and engine ops may read at most one PSUM operand --
hence the yu staging copy before the softmax normalization multiply, and
the PE broadcast matmul for 1/denominator).  fp8 matmul weight rows are
padded to 68 bytes (4-byte ISA alignment).

Emission is software-pipelined: engines execute their streams in order, so
QKV work for chunk qc+1 is interleaved into the attention pair-loop of
chunk qc at pair granularity, output-projection tiles for qc-1 drip at
head-pair boundaries (PSUM-ring-safe), and each head-pair's normalization
is delayed three pairs into the next head-pair so the recip/broadcast/mul
chain overlaps fresh scores/exp work.

The ISA has ONE semaphore-wait slot per instruction; Tile emits more.
_legalize_waits() splits extras onto same-engine NoOps post-scheduling.
"""

import math

import numpy as np

import concourse.bass as bass
import concourse.mybir as mybir
import concourse.tile as tile
from concourse import bass_utils

F32 = mybir.dt.float32
F16 = mybir.dt.float16
F8 = mybir.dt.float8e4
DR = mybir.MatmulPerfMode.DoubleRow

B, T, C, H, D = 4, 2048, 1024, 16, 64
HG = C // 2          # 512 features per head group (8 heads x 64)
NT = T // 128        # 16 t-tiles
NQ = T // 512        # 4 query/t chunks
EPS = 1.1920928955078125e-07
SCALE = 1.0 / math.sqrt(D)  # 0.125
EXP_BIAS = -3.0      # exp(s*SCALE + bias): keeps exp output < fp8e4 max

_wsplit_counter = [0]


def _legalize_waits(nc):
    """Split multi-wait instructions into single-wait NoOp chains."""
    n = 0
    for f in nc.m.functions:
        for bb in f.blocks:
            new_list = []
            changed = False
            for inst in bb.instructions:
                si = inst.sync_info
                if si is not None and si.on_wait and len(si.on_wait) > 1:
                    waits = list(si.on_wait)
                    for w in waits[:-1]:
                        _wsplit_counter[0] += 1
                        new_list.append(mybir.InstNoOp(
                            name=f"WSPLIT-{_wsplit_counter[0]}",
                            engine=inst.engine, ins=[], outs=[],
                            sync_info=mybir.SyncInfo(on_wait=[w], on_update=[]),
                        ))
                    si.on_wait = waits[-1:]
                    changed = True
                    n += 1
                new_list.append(inst)
            if changed:
                bb.instructions = new_list
    return n


def _interleave(main, extra):
    """Run main closures with extra closures dripped in proportionally."""
    n, m = len(main), len(extra)
    j = 0
    for i, u in enumerate(main):
        u()
        while m and j < m and j * n < m * (i + 1):
            extra[j]()
            j += 1
    while j < m:
        extra[j]()
        j += 1


def _build(lam: float, phases=(1, 2), legalize=True) -> bass.Bass:
    nc = bass.Bass("TRN2", target_bir_lowering=False, debug=False, num_devices=8)

    xb_d = nc.dram_tensor("x8T", [C, T], F8, kind="ExternalInput").ap()
    dxb_d = nc.dram_tensor("dx8T", [C, T], F8, kind="ExternalInput").ap()
    v1_d = nc.dram_tensor("v1b", [T, HG], F16, kind="ExternalInput").ap()
    wq_d = nc.dram_tensor("wq8T", [C, HG], F8, kind="ExternalInput").ap()
    dwq_d = nc.dram_tensor("dwq8T", [C, HG], F8, kind="ExternalInput").ap()
    wk_d = nc.dram_tensor("wk8T", [C, HG], F8, kind="ExternalInput").ap()
    dwk_d = nc.dram_tensor("dwk8T", [C, HG], F8, kind="ExternalInput").ap()
    wv_d = nc.dram_tensor("wv8T", [C, HG], F8, kind="ExternalInput").ap()
    dwv_d = nc.dram_tensor("dwv8T", [C, HG], F8, kind="ExternalInput").ap()
    wp_d = nc.dram_tensor("wpT", [HG, C], F16, kind="ExternalInput").ap()
    cos_d = nc.dram_tensor("cosn", [T, 32], F16, kind="ExternalInput").ap()
    sin_d = nc.dram_tensor("sinn", [T, 32], F16, kind="ExternalInput").ap()
    tri_d = nc.dram_tensor("tri01", [128, 128], F16, kind="ExternalInput").ap()
    ztri_d = nc.dram_tensor("ztri", [128, 256], F16, kind="ExternalInput").ap()
    out_d = nc.dram_tensor("out", [T, C], F16, kind="ExternalOutput").ap()

    with tile.TileContext(nc) as tc:
        with (
            tc.tile_pool(name="const", bufs=1) as const,
            tc.tile_pool(name="pers", bufs=1) as pers,
        ):
            epsc = const.tile([128, 1], F32)
            nc.vector.memset(epsc, EPS)
            ebias = const.tile([128, 1], F32)
            nc.vector.memset(ebias, EXP_BIAS)
            ones_f = const.tile([1, 64], F32)
            nc.vector.memset(ones_f, 1.0)
            ones1x64 = const.tile([1, 64], mybir.dt.float32r)
            nc.scalar.copy(out=ones1x64, in_=ones_f)

            # persistent transposed q/k (feature-major, f = 128*j + p)
            qT = pers.tile([128, 4, T], F16, name="qT", tag="qT")
            kT = pers.tile([128, 4, T], F16, name="kT", tag="kT")
            # values, kt-tile-paired for DoubleRow PV: [128k, plane, head, 65]
            # v8 = fp8(v); dv8 = fp8(v - v8).  Ones row: 1 in v8, 0 in dv8.
            vsp = [pers.tile([128, 2, 8, 68], F8, name=f"v{p}", tag=f"v{p}")
                   for p in range(NT // 2)]
            dvp = [pers.tile([128, 2, 8, 68], F8, name=f"dv{p}", tag=f"dv{p}")
                   for p in range(NT // 2)]
            for p in range(NT // 2):
                nc.vector.memset(vsp[p][:, :, :, 64:68], 0.0)
                nc.vector.memset(vsp[p][:, :, :, 64:65], 1.0)
                nc.gpsimd.memset(dvp[p][:, :, :, 64:68], 0.0)
            # fp8 probability ring (2 planes = one kt pair, 2 heads x 512 q)
            ptr = [pers.tile([128, 2, 1024], F8, name=f"pt{r}", tag=f"pt{r}")
                   for r in range(6)]
            for r in range(6):
                eng = nc.vector if r % 2 == 0 else nc.gpsimd
                eng.memset(ptr[r], 0.0)

            with (
                tc.tile_pool(name="p1", bufs=1) as p1,
                tc.tile_pool(name="p1ps", bufs=1, space="PSUM") as p1ps,
                tc.tile_pool(name="p2", bufs=1) as p2,
            ):
                # -------- startup: x chunk 0 first, then weights ---------
                xT_tiles = {}

                def load_x(tc4):
                    if tc4 >= NQ or tc4 in xT_tiles:
                        return
                    xt = p1.tile([128, 8, 512], F8, name="xT", tag="xT",
                                 bufs=4)
                    dxt = p1.tile([128, 8, 512], F8, name="dxT", tag="dxT",
                                  bufs=4)
                    t0 = tc4 * 512
                    for ts in range(4):
                        nc.sync.dma_start(
                            out=xt[:, :, ts * 128:(ts + 1) * 128],
                            in_=xb_d[:, t0 + ts * 128:t0 + (ts + 1) * 128]
                            .rearrange("(c p) t -> p c t", p=128))
                        nc.sync.dma_start(
                            out=dxt[:, :, ts * 128:(ts + 1) * 128],
                            in_=dxb_d[:, t0 + ts * 128:t0 + (ts + 1) * 128]
                            .rearrange("(c p) t -> p c t", p=128))
                    xT_tiles[tc4] = (xt, dxt)

                xt0 = p1.tile([128, 8, 512], F8, name="xT", tag="xT", bufs=4)
                dxt0 = p1.tile([128, 8, 512], F8, name="dxT", tag="dxT",
                               bufs=4)
                wq_sb = p1.tile([128, 2, 8, HG], F8)
                wk_sb = p1.tile([128, 2, 8, HG], F8)
                wv_sb = p1.tile([128, 2, 8, HG], F8)
                # x/dx slice 0 first (feeds the first Ldweights), then the wq
                # pair (first matmul), then remaining slices stream behind.
                nc.sync.dma_start(
                    out=xt0[:, :, 0:128],
                    in_=xb_d[:, 0:128].rearrange("(c p) t -> p c t", p=128))
                nc.sync.dma_start(
                    out=dxt0[:, :, 0:128],
                    in_=dxb_d[:, 0:128].rearrange("(c p) t -> p c t", p=128))
                nc.sync.dma_start(
                    out=wq_sb[:, 0, :, :],
                    in_=wq_d.rearrange("(c p) i -> p c i", p=128))
                nc.sync.dma_start(
                    out=wq_sb[:, 1, :, :],
                    in_=dwq_d.rearrange("(c p) i -> p c i", p=128))
                for ts in range(1, 4):
                    nc.sync.dma_start(
                        out=xt0[:, :, ts * 128:(ts + 1) * 128],
                        in_=xb_d[:, ts * 128:(ts + 1) * 128].rearrange(
                            "(c p) t -> p c t", p=128))
                    nc.sync.dma_start(
                        out=dxt0[:, :, ts * 128:(ts + 1) * 128],
                        in_=dxb_d[:, ts * 128:(ts + 1) * 128].rearrange(
                            "(c p) t -> p c t", p=128))
                xT_tiles[0] = (xt0, dxt0)
                nc.sync.dma_start(
                    out=wk_sb[:, 0, :, :],
                    in_=wk_d.rearrange("(c p) i -> p c i", p=128))
                nc.sync.dma_start(
                    out=wk_sb[:, 1, :, :],
                    in_=dwk_d.rearrange("(c p) i -> p c i", p=128))
                cos_sb = p1.tile([128, NT, 32], F16)
                nc.sync.dma_start(
                    out=cos_sb, in_=cos_d.rearrange("(n p) i -> p n i", p=128))
                sin_sb = p1.tile([128, NT, 32], F16)
                nc.sync.dma_start(
                    out=sin_sb, in_=sin_d.rearrange("(n p) i -> p n i", p=128))
                nc.sync.dma_start(
                    out=wv_sb[:, 0, :, :],
                    in_=wv_d.rearrange("(c p) i -> p c i", p=128))
                nc.sync.dma_start(
                    out=wv_sb[:, 1, :, :],
                    in_=dwv_d.rearrange("(c p) i -> p c i", p=128))
                tri01 = const.tile([128, 128], F16)
                nc.sync.dma_start(out=tri01, in_=tri_d)
                load_x(1)
                ztri = const.tile([128, 256], F16)
                nc.sync.dma_start(out=ztri, in_=ztri_d)

                w_sb = {"q": wq_sb, "k": wk_sb, "v": wv_sb}

                def p1_unit_a(tc4, which, ts):
                    """QKV matmuls + the op that releases the PSUM bank."""
                    x8t, dx8t = xT_tiles[tc4]
                    tg = tc4 * 4 + ts
                    ps = p1ps.tile([128, 512], F32, name="qkvps",
                                   tag="qkvps", bufs=2)
                    wpair = w_sb[which]
                    passes = ((x8t, 0), (dx8t, 0), (x8t, 1))
                    for pas, (xa, wi) in enumerate(passes):
                        for cc in range(4):
                            nc.tensor.matmul(
                                ps,
                                xa[:, 2 * cc:2 * cc + 2,
                                   ts * 128:(ts + 1) * 128],
                                wpair[:, wi, 2 * cc:2 * cc + 2, :],
                                start=(pas == 0 and cc == 0),
                                stop=(pas == 2 and cc == 3),
                                perf_mode=DR)
                    p3 = ps.rearrange("p (h d) -> p h d", h=8)
                    if which == "v":
                        v1t = p1.tile([128, HG], F16, name="v1t",
                                      tag="v1t", bufs=4)
                        nc.sync.dma_start(
                            out=v1t, in_=v1_d[tg * 128:(tg + 1) * 128, :])
                        vt16 = p1.tile([128, HG], F16, name="vt16",
                                       tag="vt16", bufs=2)
                        nc.vector.scalar_tensor_tensor(
                            out=vt16.rearrange("p (h d) -> p h d", h=8),
                            in0=p3, scalar=(1.0 - lam) / 64.0,
                            in1=v1t.rearrange("p (h d) -> p h d", h=8),
                            op0=mybir.AluOpType.mult,
                            op1=mybir.AluOpType.add)
                        return ("v", tg, vt16)

                    # fp16 copy releases the PSUM bank
                    qraw = p1.tile([128, 512], F16, name="qraw",
                                   tag="qraw", bufs=4)
                    nc.vector.tensor_copy(out=qraw, in_=ps)
                    return ("qk", tg, qraw, which)

                def p1_unit_b(tok):
                    if tok[0] == "v":
                        _, tg, vt16 = tok
                        v8dst = vsp[tg // 2][:, tg % 2, :, 0:64]
                        nc.gpsimd.tensor_copy(
                            out=v8dst,
                            in_=vt16.rearrange("p (h d) -> p h d", h=8))
                        nc.gpsimd.tensor_sub(
                            out=dvp[tg // 2][:, tg % 2, :, 0:64],
                            in0=vt16.rearrange("p (h d) -> p h d", h=8),
                            in1=v8dst)
                        return
                    _, tg, qraw, which = tok

                    # RMS stats
                    sqt = p1.tile([128, 512], F16, name="sqt",
                                  tag="sqt", bufs=2)
                    nc.vector.tensor_mul(out=sqt, in0=qraw, in1=qraw)
                    ssum = p1.tile([128, 8], F32, name="ssum",
                                   tag="ssum", bufs=4)
                    nc.vector.tensor_reduce(
                        ssum, sqt.rearrange("p (h d) -> p h d", h=8),
                        axis=mybir.AxisListType.X, op=mybir.AluOpType.add)
                    srt = p1.tile([128, 8], F32, name="srt", tag="srt",
                                  bufs=4)
                    nc.scalar.activation(
                        srt, ssum, mybir.ActivationFunctionType.Sqrt,
                        bias=epsc, scale=1.0 / 64.0)
                    rst = p1.tile([128, 8], F16, name="rst", tag="rst",
                                  bufs=4)
                    with nc.allow_low_precision(reason="rms scale in fp16"):
                        nc.vector.reciprocal(out=rst, in_=srt)

                    # RoPE on unnormalized q; per-head RMS scale applied last
                    rot = p1.tile([128, 512], F16, name="rot",
                                  tag="rot", bufs=2)
                    r3 = rot.rearrange("p (h d) -> p h d", h=8)
                    rc = p1.tile([128, 512], F16, name="rc", tag="rc", bufs=2)
                    rs = p1.tile([128, 512], F16, name="rs", tag="rs", bufs=2)
                    rc4 = rc.rearrange("p (h a i) -> p h a i", h=8, a=2)
                    rs4 = rs.rearrange("p (h a i) -> p h a i", h=8, a=2)
                    q4 = qraw.rearrange("p (h a i) -> p h a i", h=8, a=2)
                    c4 = cos_sb[:, tg, :].rearrange(
                        "p (o a i) -> p o a i", o=1, a=1).to_broadcast(
                        (128, 8, 2, 32))
                    s4 = sin_sb[:, tg, :].rearrange(
                        "p (o a i) -> p o a i", o=1, a=1).to_broadcast(
                        (128, 8, 2, 32))
                    nc.vector.tensor_mul(out=rc4, in0=q4, in1=c4)
                    nc.vector.tensor_mul(out=rs4, in0=q4, in1=s4)
                    # y1 = x1*c + x2*s ; y2 = x2*c - x1*s
                    nc.gpsimd.tensor_add(out=r3[:, :, 0:32],
                                         in0=rc4[:, :, 0, :],
                                         in1=rs4[:, :, 1, :])
                    nc.gpsimd.tensor_sub(out=r3[:, :, 32:64],
                                         in0=rc4[:, :, 1, :],
                                         in1=rs4[:, :, 0, :])
                    rstb = rst.rearrange("p (h o) -> p h o", o=1).to_broadcast(
                        (128, 8, 64))
                    rotn = p1.tile([128, 512], F16, name="rotn",
                                   tag="rotn", bufs=6)
                    nc.vector.tensor_mul(
                        out=rotn.rearrange("p (h d) -> p h d", h=8),
                        in0=r3, in1=rstb)

                    dstT = qT if which == "q" else kT
                    nc.sync.dma_start(
                        out=dstT[:, 0:4, tg * 128:(tg + 1) * 128],
                        in_=rotn, transpose=True)

                def p1_units(tc4):
                    units = [lambda t=tc4: load_x(t + 2)]
                    toks = []

                    def do_a(t, w, s, tk=toks):
                        tk.append(p1_unit_a(t, w, s))

                    def do_b(tk=toks):
                        if tk:
                            p1_unit_b(tk.pop(0))

                    seq = [(w, s) for w in ("q", "k", "v") for s in range(4)]
                    # stagger: emit unit i's matmuls+release before unit
                    # i-1's RMS/RoPE tail so PSUM releases lead DVE's queue
                    for i, (w, s) in enumerate(seq):
                        units.append(lambda t=tc4, w_=w, s_=s: do_a(t, w_, s_))
                        if i >= 1:
                            units.append(do_b)
                    units.append(do_b)
                    return units

                yT_all = {}
                pair_ctr = [0]

                def p2_closures(qc):
                    """Returns (main_closures, hp_boundary_indices)."""
                    yT = [p2.tile([128, 512], F16, name=f"yT{qc}_{j}",
                                  tag=f"yT{qc}_{j}", bufs=1) for j in range(4)]
                    yT_all[qc] = yT

                    def make_hp(hp):
                        pair = (2 * hp, 2 * hp + 1)
                        npair = 2 * qc + 2
                        state = {"pv": None, "pending": []}

                        def alloc_pv():
                            state["pv"] = {h: p1ps.tile([68, 512], F32,
                                                        name="pv", tag="pv",
                                                        bufs=2)
                                           for h in pair}

                        def emit_pv(idx):
                            pt, pi, a0p = state["pending"][idx]
                            for h in pair:
                                o = 512 * (h % 2)
                                for pas, vt in ((0, vsp[pi]), (1, dvp[pi])):
                                    nc.tensor.matmul(
                                        state["pv"][h][:, a0p:512],
                                        vt[:, :, h, :],
                                        pt[:, :, o + a0p:o + 512],
                                        start=(pi == 0 and pas == 0),
                                        stop=(pi == npair - 1 and pas == 1),
                                        perf_mode=DR)

                        def do_pair(pidx):
                            if pidx == 0:
                                alloc_pv()
                            kt0, kt1 = 2 * pidx, 2 * pidx + 1
                            m0 = kt0 - 4 * qc
                            a0p = 128 * m0 if m0 >= 1 else 0
                            pt = ptr[pair_ctr[0] % 6]
                            pair_ctr[0] += 1
                            for j, kt in ((0, kt0), (1, kt1)):
                                m = kt - 4 * qc
                                a0s = 128 * m if m >= 1 else 0
                                st_t = p1ps.tile([128, 1024], F32, name="st",
                                                 tag="st", bufs=2)
                                for h in pair:
                                    b0 = 64 * (h % 2)
                                    o = 512 * (h % 2)
                                    nc.tensor.matmul(
                                        st_t[:, o + a0s:o + 512],
                                        kT[b0:b0 + 64, hp,
                                           kt * 128:(kt + 1) * 128],
                                        qT[b0:b0 + 64, hp,
                                           qc * 512 + a0s:(qc + 1) * 512],
                                        start=True, stop=True)
                                ptj = pt[:, j, :]
                                pt2 = ptj.rearrange("p (h q) -> p h q", h=2)
                                st2 = st_t.rearrange("p (h q) -> p h q", h=2)
                                if m <= 0:
                                    nc.scalar.activation(
                                        ptj, st_t,
                                        mybir.ActivationFunctionType.Exp,
                                        scale=SCALE, bias=ebias)
                                else:
                                    w0 = 128 * m
                                    nc.scalar.activation(
                                        pt2[:, :, w0:512], st2[:, :, w0:512],
                                        mybir.ActivationFunctionType.Exp,
                                        scale=SCALE, bias=ebias)
                                if m in (0, 2):
                                    nc.gpsimd.tensor_mul(
                                        out=pt2[:, :, a0s:a0s + 128],
                                        in0=pt2[:, :, a0s:a0s + 128],
                                        in1=tri01.rearrange(
                                            "p (o q) -> p o q",
                                            o=1).to_broadcast((128, 2, 128)))
                                elif m in (1, 3):
                                    nc.gpsimd.tensor_mul(
                                        out=pt2[:, :, a0p:a0p + 256],
                                        in0=pt2[:, :, a0p:a0p + 256],
                                        in1=ztri.rearrange(
                                            "p (o q) -> p o q",
                                            o=1).to_broadcast((128, 2, 256)))
                            state["pending"].append((pt, pidx, a0p))
                            if len(state["pending"]) >= 4:
                                emit_pv(len(state["pending"]) - 4)
                            if pidx == npair - 1:
                                for k in range(min(3, len(state["pending"])),
                                               0, -1):
                                    emit_pv(len(state["pending"]) - k)

                        def do_norm():
                            yus = {}
                            for h in pair:
                                yu = p2.tile([65, 512], F16, name="yu",
                                             tag="yu", bufs=4)
                                if h % 2 == 0:
                                    nc.scalar.copy(out=yu,
                                                   in_=state["pv"][h][0:65, :])
                                else:
                                    nc.vector.tensor_copy(
                                        out=yu, in_=state["pv"][h][0:65, :])
                                yus[h] = yu
                            for h in pair:
                                b0 = 64 * (h % 2)
                                yu = yus[h]
                                rec = p2.tile([1, 512], mybir.dt.float32r,
                                              name="rec", tag="rec", bufs=4)
                                with nc.allow_low_precision(
                                        reason="softmax denom recip"):
                                    nc.vector.reciprocal(
                                        out=rec, in_=yu[64:65, :])
                                rb = p1ps.tile([64, 512], F32, name="bcr",
                                               tag="pv", bufs=2)
                                nc.tensor.matmul(rb, ones1x64, rec,
                                                 start=True, stop=True)
                                nc.vector.tensor_mul(
                                    out=yT[hp][b0:b0 + 64, :],
                                    in0=yu[0:64, :], in1=rb)

                        units = [(lambda p=pidx: do_pair(p))
                                 for pidx in range(npair)]
                        units.append(do_norm)
                        return units

                    # Delay each hp's norm unit until after the next hp's
                    # first pair, so the recip->bcr->mul chain overlaps with
                    # fresh scores/exp work instead of stalling the engines
                    # at the head-pair boundary.
                    d = 3 if qc >= 1 else 2
                    main = []
                    bounds = []
                    held_norm = None
                    for hp in range(4):
                        units = make_hp(hp)
                        pairs_u, norm_u = units[:-1], units[-1]
                        main.extend(pairs_u[:d])
                        if held_norm is not None:
                            main.append(held_norm)
                            bounds.append(len(main))
                        main.extend(pairs_u[d:])
                        held_norm = norm_u
                    main.append(held_norm)
                    bounds.append(len(main))
                    return main, bounds

                wp_sb = p2.tile([128, 4, C], F16)

                def proj_unit(qc, tsub, jc):
                    yT = yT_all[qc]
                    pr = p1ps.tile([128, 512], F32, name="pr", tag="qkvps",
                                   bufs=2)
                    for ft in range(4):
                        nc.tensor.matmul(
                            pr,
                            yT[ft][:, tsub * 128:(tsub + 1) * 128],
                            wp_sb[:, ft, jc * 512:(jc + 1) * 512],
                            start=(ft == 0), stop=(ft == 3))
                    osb = p2.tile([128, 512], F16, name="osb", tag="osb",
                                  bufs=4)
                    nc.vector.tensor_copy(out=osb, in_=pr)
                    r0 = qc * 512 + tsub * 128
                    nc.sync.dma_start(
                        out=out_d[r0:r0 + 128, jc * 512:(jc + 1) * 512],
                        in_=osb)

                def proj_units(qc):
                    units = []
                    for tsub in range(4):
                        for jc in range(2):
                            units.append(
                                lambda q=qc, t=tsub, j=jc: proj_unit(q, t, j))
                    return units

                # ---------------- emission schedule ----------------
                if 1 in phases and 2 in phases:
                    for u in p1_units(0):
                        u()
                    nc.sync.dma_start(
                        out=wp_sb, in_=wp_d.rearrange("(c p) j -> p c j",
                                                      p=128))
                    for qc in range(NQ):
                        main, bounds = p2_closures(qc)
                        drip = p1_units(qc + 1) if qc + 1 < NQ else []
                        # proj(0)+proj(1) at p2(2) hp boundaries; proj(2) is
                        # fine-dripped through p2(3) (qkvps ring is idle
                        # there); proj(3) runs at the tail.
                        pu = (proj_units(0) + proj_units(1)) if qc == 2 else []
                        if qc == 3:
                            drip = proj_units(2)
                        pb = {}
                        for bi, b in enumerate(bounds):
                            k = len(pu) // len(bounds)
                            pb[b] = pu[k * bi:k * (bi + 1)]
                        main2 = []
                        for i, u in enumerate(main):
                            main2.append(u)
                            for x in pb.get(i + 1, []):
                                main2.append(x)
                        _interleave(main2, drip)
                    for u in proj_units(NQ - 1):
                        u()
                elif 1 in phases:
                    for tc4 in range(NQ):
                        for u in p1_units(tc4):
                            u()

    if legalize:
        _legalize_waits(nc)
    return nc


def _host_tables():
    inv_freq = 1.0 / (10000.0 ** (np.arange(0, D, 2, dtype=np.float32) / D))
    t = np.arange(T, dtype=np.float32)
    freqs = np.outer(t, inv_freq).astype(np.float32)      # (T, 32)
    cos16 = np.cos(freqs).astype(np.float16)
    sin16 = np.sin(freqs).astype(np.float16)
    p = np.arange(128)[:, None]
    f = np.arange(128)[None, :]
    tri = (p <= f).astype(np.float16)                     # (128, 128)
    ztri = np.concatenate([np.zeros((128, 128), np.float16), tri], axis=1)
    return cos16, sin16, tri, ztri


_CACHE = {}


def kernel(x, v1, wq, wk, wv, wproj, lamb):
    x = np.asarray(x, dtype=np.float32)
    v1 = np.asarray(v1, dtype=np.float32)
    wq = np.asarray(wq, dtype=np.float32)
    wk = np.asarray(wk, dtype=np.float32)
    wv = np.asarray(wv, dtype=np.float32)
    wproj = np.asarray(wproj, dtype=np.float32)
    lam = float(np.asarray(lamb))

    cosn, sinn, tri, ztri = _host_tables()
    import ml_dtypes
    f8 = ml_dtypes.float8_e4m3

    def q3(a, scale):
        a = np.asarray(a * scale, dtype=np.float32)
        a8 = a.astype(f8)
        da8 = (a - a8.astype(np.float32)).astype(f8)
        return a8, da8

    key = lam
    if key not in _CACHE:
        _CACHE[key] = _build(lam)
    nc = _CACHE[key]

    in_maps = []
    for core in range(8):
        b, hg = core // 2, core % 2
        sl = slice(hg * HG, (hg + 1) * HG)
        x8, dx8 = q3(x[b].T, 1.0)
        wq8, dwq8 = q3(wq[sl, :].T, 64.0)
        wk8, dwk8 = q3(wk[sl, :].T, 64.0)
        wv8, dwv8 = q3(wv[sl, :].T, 64.0)
        in_maps.append({
            "x8T": np.ascontiguousarray(x8),
            "dx8T": np.ascontiguousarray(dx8),
            "v1b": np.ascontiguousarray(
                (lam * v1[b][:, sl]).astype(np.float16)),
            "wq8T": np.ascontiguousarray(wq8),
            "dwq8T": np.ascontiguousarray(dwq8),
            "wk8T": np.ascontiguousarray(wk8),
            "dwk8T": np.ascontiguousarray(dwk8),
            "wv8T": np.ascontiguousarray(wv8),
            "dwv8T": np.ascontiguousarray(dwv8),
            "wpT": np.ascontiguousarray(wproj[:, sl].T.astype(np.float16)),
            "cosn": cosn,
            "sinn": sinn,
            "tri01": tri,
            "ztri": ztri,
        })

    res = bass_utils.run_bass_kernel_spmd(nc, in_maps, core_ids=list(range(8)))
    y = np.empty((B, T, C), dtype=np.float32)
    for b in range(B):
        y[b] = (res.results[2 * b]["out"].astype(np.float32)
                + res.results[2 * b + 1]["out"].astype(np.float32))
    return (y, v1)


# revision 40
# speedup vs baseline: 1.0048x; 1.0048x over previous
"""Causal self-attention (B=4, T=2048, C=1024, H=16, D=64) on 8 TRN2 cores.

Sharding: core = 2*b + hg  (b = batch 0..3, hg = head-group 0..1 of 8 heads).
Each core computes its batch's QKV projections for its 8 heads (tensor
parallel over wq/wk/wv rows), RMSNorm+RoPE, causal attention, and a partial
output projection over its head-group's wproj columns. The two partials per
batch are summed on the host.

v7: fp8e4 DoubleRow matmuls with error-feedback for QKV and PV.
QKV runs three accumulating DoubleRow passes -- x8@w8 + dx8@w8 + x8@dw8 --
where dx8/dw8 are the fp8 quantization residuals (prepared on host, w
pre-scaled x64: the q/k scale cancels in RMSNorm, the v scale folds into the
value blend).  This cuts QKV PE time 25% below fp16 at ~0.1% extra error.
Values are stored as v8 + dv8 planes; PV runs two DoubleRow passes per
kt-pair (pt8 x v8, pt8 x dv8), contraction 256, with the ones-row riding in
v8 (zeros in dv8) for softmax denominators.  Probabilities go to fp8
directly from the ACT exp (bias -3 keeps exp inside fp8 range and cancels
in normalization).  Scores stay fp16 (qT/kT via DMA XBAR transpose);
RMSNorm applies AFTER RoPE (rotation commutes with the per-head scale).
Diagonal-strip exps cover both heads via one strided ACT instruction;
triangle masks use [tri]/[0|tri] tables on gpsimd (SBUF-only: TRN2 gpsimd
cannot access PSUM, and engine ops may read at most one PSUM operand --
hence the yu staging copy before the softmax normalization multiply, and
the PE broadcast matmul for 1/denominator).  fp8 matmul weight rows are
padded to 68 bytes (4-byte ISA alignment).

Emission is software-pipelined: engines execute their streams in order, so
QKV work for chunk qc+1 is interleaved into the attention pair-loop of
chunk qc at pair granularity, output-projection tiles for qc-1 drip at
head-pair boundaries (PSUM-ring-safe), and each head-pair's normalization
is delayed three pairs into the next head-pair so the recip/broadcast/mul
chain overlaps fresh scores/exp work.

The ISA has ONE semaphore-wait slot per instruction; Tile emits more.
_legalize_waits() splits extras onto same-engine NoOps post-scheduling.
"""

import math

import numpy as np

import concourse.bass as bass
import concourse.mybir as mybir
import concourse.tile as tile
from concourse import bass_utils

F32 = mybir.dt.float32
F16 = mybir.dt.float16
F8 = mybir.dt.float8e4
DR = mybir.MatmulPerfMode.DoubleRow

B, T, C, H, D = 4, 2048, 1024, 16, 64
HG = C // 2          # 512 features per head group (8 heads x 64)
NT = T // 128        # 16 t-tiles
NQ = T // 512        # 4 query/t chunks
EPS = 1.1920928955078125e-07
SCALE = 1.0 / math.sqrt(D)  # 0.125
EXP_BIAS = -3.0      # exp(s*SCALE + bias): keeps exp output < fp8e4 max

_wsplit_counter = [0]


def _legalize_waits(nc):
    """Split multi-wait instructions into single-wait NoOp chains."""
    n = 0
    for f in nc.m.functions:
        for bb in f.blocks:
            new_list = []
            changed = False
            for inst in bb.instructions:
                si = inst.sync_info
                if si is not None and si.on_wait and len(si.on_wait) > 1:
                    waits = list(si.on_wait)
                    for w in waits[:-1]:
                        _wsplit_counter[0] += 1
                        new_list.append(mybir.InstNoOp(
                            name=f"WSPLIT-{_wsplit_counter[0]}",
                            engine=inst.engine, ins=[], outs=[],
                            sync_info=mybir.SyncInfo(on_wait=[w], on_update=[]),
                        ))
                    si.on_wait = waits[-1:]
                    changed = True
                    n += 1
                new_list.append(inst)
            if changed:
                bb.instructions = new_list
    return n


def _interleave(main, extra):
    """Run main closures with extra closures dripped in proportionally."""
    n, m = len(main), len(extra)
    j = 0
    for i, u in enumerate(main):
        u()
        while m and j < m and j * n < m * (i + 1):
            extra[j]()
            j += 1
    while j < m:
        extra[j]()
        j += 1


def _build(lam: float, phases=(1, 2), legalize=True) -> bass.Bass:
    nc = bass.Bass("TRN2", target_bir_lowering=False, debug=False, num_devices=8)

    xb_d = nc.dram_tensor("x8T", [C, T], F8, kind="ExternalInput").ap()
    dxb_d = nc.dram_tensor("dx8T", [C, T], F8, kind="ExternalInput").ap()
    v1_d = nc.dram_tensor("v1b", [T, HG], F16, kind="ExternalInput").ap()
    wq_d = nc.dram_tensor("wq8T", [C, HG], F8, kind="ExternalInput").ap()
    dwq_d = nc.dram_tensor("dwq8T", [C, HG], F8, kind="ExternalInput").ap()
    wk_d = nc.dram_tensor("wk8T", [C, HG], F8, kind="ExternalInput").ap()
    dwk_d = nc.dram_tensor("dwk8T", [C, HG], F8, kind="ExternalInput").ap()
    wv_d = nc.dram_tensor("wv8T", [C, HG], F8, kind="ExternalInput").ap()
    dwv_d = nc.dram_tensor("dwv8T", [C, HG], F8, kind="ExternalInput").ap()
    wp_d = nc.dram_tensor("wpT", [HG, C], F16, kind="ExternalInput").ap()
    cos_d = nc.dram_tensor("cosn", [T, 32], F16, kind="ExternalInput").ap()
    sin_d = nc.dram_tensor("sinn", [T, 32], F16, kind="ExternalInput").ap()
    tri_d = nc.dram_tensor("tri01", [128, 128], F16, kind="ExternalInput").ap()
    ztri_d = nc.dram_tensor("ztri", [128, 256], F16, kind="ExternalInput").ap()
    out_d = nc.dram_tensor("out", [T, C], F16, kind="ExternalOutput").ap()

    with tile.TileContext(nc) as tc:
        with (
            tc.tile_pool(name="const", bufs=1) as const,
            tc.tile_pool(name="pers", bufs=1) as pers,
        ):
            epsc = const.tile([128, 1], F32)
            nc.vector.memset(epsc, EPS)
            ebias = const.tile([128, 1], F32)
            nc.vector.memset(ebias, EXP_BIAS)
            ones_f = const.tile([1, 64], F32)
            nc.vector.memset(ones_f, 1.0)
            ones1x64 = const.tile([1, 64], mybir.dt.float32r)
            nc.scalar.copy(out=ones1x64, in_=ones_f)

            # persistent transposed q/k (feature-major, f = 128*j + p)
            qT = pers.tile([128, 4, T], F16, name="qT", tag="qT")
            kT = pers.tile([128, 4, T], F16, name="kT", tag="kT")
            # values, kt-tile-paired for DoubleRow PV: [128k, plane, head, 65]
            # v8 = fp8(v); dv8 = fp8(v - v8).  Ones row: 1 in v8, 0 in dv8.
            vsp = [pers.tile([128, 2, 8, 68], F8, name=f"v{p}", tag=f"v{p}")
                   for p in range(NT // 2)]
            dvp = [pers.tile([128, 2, 8, 68], F8, name=f"dv{p}", tag=f"dv{p}")
                   for p in range(NT // 2)]
            for p in range(NT // 2):
                nc.vector.memset(vsp[p][:, :, :, 64:68], 0.0)
                nc.vector.memset(vsp[p][:, :, :, 64:65], 1.0)
                nc.gpsimd.memset(dvp[p][:, :, :, 64:68], 0.0)
            # fp8 probability ring (2 planes = one kt pair, 2 heads x 512 q)
            ptr = [pers.tile([128, 2, 1024], F8, name=f"pt{r}", tag=f"pt{r}")
                   for r in range(6)]
            for r in range(6):
                eng = nc.vector if r % 2 == 0 else nc.gpsimd
                eng.memset(ptr[r], 0.0)

            with (
                tc.tile_pool(name="p1", bufs=1) as p1,
                tc.tile_pool(name="p1ps", bufs=1, space="PSUM") as p1ps,
                tc.tile_pool(name="p2", bufs=1) as p2,
            ):
                # -------- startup: x chunk 0 first, then weights ---------
                xT_tiles = {}

                def load_x(tc4):
                    if tc4 >= NQ or tc4 in xT_tiles:
                        return
                    xt = p1.tile([128, 8, 512], F8, name="xT", tag="xT",
                                 bufs=4)
                    dxt = p1.tile([128, 8, 512], F8, name="dxT", tag="dxT",
                                  bufs=4)
                    t0 = tc4 * 512
                    for ts in range(4):
                        nc.sync.dma_start(
                            out=xt[:, :, ts * 128:(ts + 1) * 128],
                            in_=xb_d[:, t0 + ts * 128:t0 + (ts + 1) * 128]
                            .rearrange("(c p) t -> p c t", p=128))
                        nc.sync.dma_start(
                            out=dxt[:, :, ts * 128:(ts + 1) * 128],
                            in_=dxb_d[:, t0 + ts * 128:t0 + (ts + 1) * 128]
                            .rearrange("(c p) t -> p c t", p=128))
                    xT_tiles[tc4] = (xt, dxt)

                xt0 = p1.tile([128, 8, 512], F8, name="xT", tag="xT", bufs=4)
                dxt0 = p1.tile([128, 8, 512], F8, name="dxT", tag="dxT",
                               bufs=4)
                wq_sb = p1.tile([128, 2, 8, HG], F8)
                wk_sb = p1.tile([128, 2, 8, HG], F8)
                wv_sb = p1.tile([128, 2, 8, HG], F8)
                # x/dx slice 0 first (feeds the first Ldweights), then the wq
                # pair (first matmul), then remaining slices stream behind.
                nc.sync.dma_start(
                    out=xt0[:, :, 0:128],
                    in_=xb_d[:, 0:128].rearrange("(c p) t -> p c t", p=128))
                nc.sync.dma_start(
                    out=dxt0[:, :, 0:128],
                    in_=dxb_d[:, 0:128].rearrange("(c p) t -> p c t", p=128))
                nc.sync.dma_start(
                    out=wq_sb[:, 0, :, :],
                    in_=wq_d.rearrange("(c p) i -> p c i", p=128))
                nc.sync.dma_start(
                    out=wq_sb[:, 1, :, :],
                    in_=dwq_d.rearrange("(c p) i -> p c i", p=128))
                for ts in range(1, 4):
                    nc.sync.dma_start(
                        out=xt0[:, :, ts * 128:(ts + 1) * 128],
                        in_=xb_d[:, ts * 128:(ts + 1) * 128].rearrange(
                            "(c p) t -> p c t", p=128))
                    nc.sync.dma_start(
                        out=dxt0[:, :, ts * 128:(ts + 1) * 128],
                        in_=dxb_d[:, ts * 128:(ts + 1) * 128].rearrange(
                            "(c p) t -> p c t", p=128))
                xT_tiles[0] = (xt0, dxt0)
                nc.sync.dma_start(
                    out=wk_sb[:, 0, :, :],
                    in_=wk_d.rearrange("(c p) i -> p c i", p=128))
                nc.sync.dma_start(
                    out=wk_sb[:, 1, :, :],
                    in_=dwk_d.rearrange("(c p) i -> p c i", p=128))
                cos_sb = p1.tile([128, NT, 32], F16)
                nc.sync.dma_start(
                    out=cos_sb, in_=cos_d.rearrange("(n p) i -> p n i", p=128))
                sin_sb = p1.tile([128, NT, 32], F16)
                nc.sync.dma_start(
                    out=sin_sb, in_=sin_d.rearrange("(n p) i -> p n i", p=128))
                nc.sync.dma_start(
                    out=wv_sb[:, 0, :, :],
                    in_=wv_d.rearrange("(c p) i -> p c i", p=128))
                nc.sync.dma_start(
                    out=wv_sb[:, 1, :, :],
                    in_=dwv_d.rearrange("(c p) i -> p c i", p=128))
                tri01 = const.tile([128, 128], F16)
                nc.sync.dma_start(out=tri01, in_=tri_d)
                load_x(1)
                ztri = const.tile([128, 256], F16)
                nc.sync.dma_start(out=ztri, in_=ztri_d)

                w_sb = {"q": wq_sb, "k": wk_sb, "v": wv_sb}

                def p1_unit_a(tc4, which, ts):
                    """QKV matmuls + the op that releases the PSUM bank."""
                    x8t, dx8t = xT_tiles[tc4]
                    tg = tc4 * 4 + ts
                    ps = p1ps.tile([128, 512], F32, name="qkvps",
                                   tag="qkvps", bufs=2)
                    wpair = w_sb[which]
                    passes = ((x8t, 0), (dx8t, 0), (x8t, 1))
                    for pas, (xa, wi) in enumerate(passes):
                        for cc in range(4):
                            nc.tensor.matmul(
                                ps,
                                xa[:, 2 * cc:2 * cc + 2,
                                   ts * 128:(ts + 1) * 128],
                                wpair[:, wi, 2 * cc:2 * cc + 2, :],
                                start=(pas == 0 and cc == 0),
                                stop=(pas == 2 and cc == 3),
                                perf_mode=DR)
                    p3 = ps.rearrange("p (h d) -> p h d", h=8)
                    if which == "v":
                        v1t = p1.tile([128, HG], F16, name="v1t",
                                      tag="v1t", bufs=4)
                        nc.sync.dma_start(
                            out=v1t, in_=v1_d[tg * 128:(tg + 1) * 128, :])
                        vt16 = p1.tile([128, HG], F16, name="vt16",
                                       tag="vt16", bufs=2)
                        nc.vector.scalar_tensor_tensor(
                            out=vt16.rearrange("p (h d) -> p h d", h=8),
                            in0=p3, scalar=(1.0 - lam) / 64.0,
                            in1=v1t.rearrange("p (h d) -> p h d", h=8),
                            op0=mybir.AluOpType.mult,
                            op1=mybir.AluOpType.add)
                        return ("v", tg, vt16)

                    # fp16 copy releases the PSUM bank
                    qraw = p1.tile([128, 512], F16, name="qraw",
                                   tag="qraw", bufs=4)
                    nc.vector.tensor_copy(out=qraw, in_=ps)
                    return ("qk", tg, qraw, which)

                def p1_unit_b(tok):
                    if tok[0] == "v":
                        _, tg, vt16 = tok
                        v8dst = vsp[tg // 2][:, tg % 2, :, 0:64]
                        nc.gpsimd.tensor_copy(
                            out=v8dst,
                            in_=vt16.rearrange("p (h d) -> p h d", h=8))
                        nc.gpsimd.tensor_sub(
                            out=dvp[tg // 2][:, tg % 2, :, 0:64],
                            in0=vt16.rearrange("p (h d) -> p h d", h=8),
                            in1=v8dst)
                        return
                    _, tg, qraw, which = tok

                    # RMS stats
                    sqt = p1.tile([128, 512], F16, name="sqt",
                                  tag="sqt", bufs=2)
                    nc.vector.tensor_mul(out=sqt, in0=qraw, in1=qraw)
                    ssum = p1.tile([128, 8], F32, name="ssum",
                                   tag="ssum", bufs=4)
                    nc.vector.tensor_reduce(
                        ssum, sqt.rearrange("p (h d) -> p h d", h=8),
                        axis=mybir.AxisListType.X, op=mybir.AluOpType.add)
                    srt = p1.tile([128, 8], F32, name="srt", tag="srt",
                                  bufs=4)
                    nc.scalar.activation(
                        srt, ssum, mybir.ActivationFunctionType.Sqrt,
                        bias=epsc, scale=1.0 / 64.0)
                    rst = p1.tile([128, 8], F16, name="rst", tag="rst",
                                  bufs=4)
                    with nc.allow_low_precision(reason="rms scale in fp16"):
                        nc.vector.reciprocal(out=rst, in_=srt)

                    # RoPE on unnormalized q; per-head RMS scale applied last
                    rot = p1.tile([128, 512], F16, name="rot",
                                  tag="rot", bufs=2)
                    r3 = rot.rearrange("p (h d) -> p h d", h=8)
                    rc = p1.tile([128, 512], F16, name="rc", tag="rc", bufs=2)
                    rs = p1.tile([128, 512], F16, name="rs", tag="rs", bufs=2)
                    rc4 = rc.rearrange("p (h a i) -> p h a i", h=8, a=2)
                    rs4 = rs.rearrange("p (h a i) -> p h a i", h=8, a=2)
                    q4 = qraw.rearrange("p (h a i) -> p h a i", h=8, a=2)
                    c4 = cos_sb[:, tg, :].rearrange(
                        "p (o a i) -> p o a i", o=1, a=1).to_broadcast(
                        (128, 8, 2, 32))
                    s4 = sin_sb[:, tg, :].rearrange(
                        "p (o a i) -> p o a i", o=1, a=1).to_broadcast(
                        (128, 8, 2, 32))
                    nc.vector.tensor_mul(out=rc4, in0=q4, in1=c4)
                    nc.vector.tensor_mul(out=rs4, in0=q4, in1=s4)
                    # y1 = x1*c + x2*s ; y2 = x2*c - x1*s
                    nc.gpsimd.tensor_add(out=r3[:, :, 0:32],
                                         in0=rc4[:, :, 0, :],
                                         in1=rs4[:, :, 1, :])
                    nc.gpsimd.tensor_sub(out=r3[:, :, 32:64],
                                         in0=rc4[:, :, 1, :],
                                         in1=rs4[:, :, 0, :])
                    rstb = rst.rearrange("p (h o) -> p h o", o=1).to_broadcast(
                        (128, 8, 64))
                    rotn = p1.tile([128, 512], F16, name="rotn",
                                   tag="rotn", bufs=6)
                    nc.vector.tensor_mul(
                        out=rotn.rearrange("p (h d) -> p h d", h=8),
                        in0=r3, in1=rstb)

                    dstT = qT if which == "q" else kT
                    nc.sync.dma_start(
                        out=dstT[:, 0:4, tg * 128:(tg + 1) * 128],
                        in_=rotn, transpose=True)

                def p1_units(tc4):
                    units = [lambda t=tc4: load_x(t + 2)]
                    toks = []

                    def do_a(t, w, s, tk=toks):
                        tk.append(p1_unit_a(t, w, s))

                    def do_b(tk=toks):
                        if tk:
                            p1_unit_b(tk.pop(0))

                    seq = [(w, s) for w in ("q", "k", "v") for s in range(4)]
                    for i, (w, s) in enumerate(seq):
                        units.append(lambda t=tc4, w_=w, s_=s: do_a(t, w_, s_))
                        units.append(do_b)
                    return units

                yT_all = {}
                pair_ctr = [0]

                def p2_closures(qc):
                    """Returns (main_closures, hp_boundary_indices)."""
                    yT = [p2.tile([128, 512], F16, name=f"yT{qc}_{j}",
                                  tag=f"yT{qc}_{j}", bufs=1) for j in range(4)]
                    yT_all[qc] = yT

                    def make_hp(hp):
                        pair = (2 * hp, 2 * hp + 1)
                        npair = 2 * qc + 2
                        state = {"pv": None, "pending": []}

                        def alloc_pv():
                            state["pv"] = {h: p1ps.tile([68, 512], F32,
                                                        name="pv", tag="pv",
                                                        bufs=2)
                                           for h in pair}

                        def emit_pv(idx):
                            pt, pi, a0p = state["pending"][idx]
                            for h in pair:
                                o = 512 * (h % 2)
                                for pas, vt in ((0, vsp[pi]), (1, dvp[pi])):
                                    nc.tensor.matmul(
                                        state["pv"][h][:, a0p:512],
                                        vt[:, :, h, :],
                                        pt[:, :, o + a0p:o + 512],
                                        start=(pi == 0 and pas == 0),
                                        stop=(pi == npair - 1 and pas == 1),
                                        perf_mode=DR)

                        def do_pair(pidx):
                            if pidx == 0:
                                alloc_pv()
                            kt0, kt1 = 2 * pidx, 2 * pidx + 1
                            m0 = kt0 - 4 * qc
                            a0p = 128 * m0 if m0 >= 1 else 0
                            pt = ptr[pair_ctr[0] % 6]
                            pair_ctr[0] += 1
                            for j, kt in ((0, kt0), (1, kt1)):
                                m = kt - 4 * qc
                                a0s = 128 * m if m >= 1 else 0
                                st_t = p1ps.tile([128, 1024], F32, name="st",
                                                 tag="st", bufs=2)
                                for h in pair:
                                    b0 = 64 * (h % 2)
                                    o = 512 * (h % 2)
                                    nc.tensor.matmul(
                                        st_t[:, o + a0s:o + 512],
                                        kT[b0:b0 + 64, hp,
                                           kt * 128:(kt + 1) * 128],
                                        qT[b0:b0 + 64, hp,
                                           qc * 512 + a0s:(qc + 1) * 512],
                                        start=True, stop=True)
                                ptj = pt[:, j, :]
                                pt2 = ptj.rearrange("p (h q) -> p h q", h=2)
                                st2 = st_t.rearrange("p (h q) -> p h q", h=2)
                                if m <= 0:
                                    nc.scalar.activation(
                                        ptj, st_t,
                                        mybir.ActivationFunctionType.Exp,
                                        scale=SCALE, bias=ebias)
                                else:
                                    w0 = 128 * m
                                    nc.scalar.activation(
                                        pt2[:, :, w0:512], st2[:, :, w0:512],
                                        mybir.ActivationFunctionType.Exp,
                                        scale=SCALE, bias=ebias)
                                if m in (0, 2):
                                    nc.gpsimd.tensor_mul(
                                        out=pt2[:, :, a0s:a0s + 128],
                                        in0=pt2[:, :, a0s:a0s + 128],
                                        in1=tri01.rearrange(
                                            "p (o q) -> p o q",
                                            o=1).to_broadcast((128, 2, 128)))
                                elif m in (1, 3):
                                    nc.gpsimd.tensor_mul(
                                        out=pt2[:, :, a0p:a0p + 256],
                                        in0=pt2[:, :, a0p:a0p + 256],
                                        in1=ztri.rearrange(
                                            "p (o q) -> p o q",
                                            o=1).to_broadcast((128, 2, 256)))
                            state["pending"].append((pt, pidx, a0p))
                            if len(state["pending"]) >= 4:
                                emit_pv(len(state["pending"]) - 4)
                            if pidx == npair - 1:
                                for k in range(min(3, len(state["pending"])),
                                               0, -1):
                                    emit_pv(len(state["pending"]) - k)

                        def do_norm():
                            yus = {}
                            for h in pair:
                                yu = p2.tile([65, 512], F16, name="yu",
                                             tag="yu", bufs=4)
                                if h % 2 == 0:
                                    nc.scalar.copy(out=yu,
                                                   in_=state["pv"][h][0:65, :])
                                else:
                                    nc.vector.tensor_copy(
                                        out=yu, in_=state["pv"][h][0:65, :])
                                yus[h] = yu
                            for h in pair:
                                b0 = 64 * (h % 2)
                                yu = yus[h]
                                rec = p2.tile([1, 512], mybir.dt.float32r,
                                              name="rec", tag="rec", bufs=4)
                                with nc.allow_low_precision(
                                        reason="softmax denom recip"):
                                    nc.vector.reciprocal(
                                        out=rec, in_=yu[64:65, :])
                                rb = p1ps.tile([64, 512], F32, name="bcr",
                                               tag="pv", bufs=2)
                                nc.tensor.matmul(rb, ones1x64, rec,
                                                 start=True, stop=True)
                                nc.vector.tensor_mul(
                                    out=yT[hp][b0:b0 + 64, :],
                                    in0=yu[0:64, :], in1=rb)

                        units = [(lambda p=pidx: do_pair(p))
                                 for pidx in range(npair)]
                        units.append(do_norm)
                        return units

                    # Delay each hp's norm unit until after the next hp's
                    # first pair, so the recip->bcr->mul chain overlaps with
                    # fresh scores/exp work instead of stalling the engines
                    # at the head-pair boundary.
                    d = 3 if qc >= 1 else 2
                    main = []
                    bounds = []
                    held_norm = None
                    for hp in range(4):
                        units = make_hp(hp)
                        pairs_u, norm_u = units[:-1], units[-1]
                        main.extend(pairs_u[:d])
                        if held_norm is not None:
                            main.append(held_norm)
                            bounds.append(len(main))
                        main.extend(pairs_u[d:])
                        held_norm = norm_u
                    main.append(held_norm)
                    bounds.append(len(main))
                    return main, bounds

                wp_sb = p2.tile([128, 4, C], F16)

                def proj_unit(qc, tsub, jc):
                    yT = yT_all[qc]
                    pr = p1ps.tile([128, 512], F32, name="pr", tag="qkvps",
                                   bufs=2)
                    for ft in range(4):
                        nc.tensor.matmul(
                            pr,
                            yT[ft][:, tsub * 128:(tsub + 1) * 128],
                            wp_sb[:, ft, jc * 512:(jc + 1) * 512],
                            start=(ft == 0), stop=(ft == 3))
                    osb = p2.tile([128, 512], F16, name="osb", tag="osb",
                                  bufs=4)
                    nc.vector.tensor_copy(out=osb, in_=pr)
                    r0 = qc * 512 + tsub * 128
                    nc.sync.dma_start(
                        out=out_d[r0:r0 + 128, jc * 512:(jc + 1) * 512],
                        in_=osb)

                def proj_units(qc):
                    units = []
                    for tsub in range(4):
                        for jc in range(2):
                            units.append(
                                lambda q=qc, t=tsub, j=jc: proj_unit(q, t, j))
                    return units

                # ---------------- emission schedule ----------------
                if 1 in phases and 2 in phases:
                    for u in p1_units(0):
                        u()
                    nc.sync.dma_start(
                        out=wp_sb, in_=wp_d.rearrange("(c p) j -> p c j",
                                                      p=128))
                    for qc in range(NQ):
                        main, bounds = p2_closures(qc)
                        drip = p1_units(qc + 1) if qc + 1 < NQ else []
                        # proj(0)+proj(1) at p2(2) hp boundaries; proj(2) is
                        # fine-dripped through p2(3) (qkvps ring is idle
                        # there); proj(3) runs at the tail.
                        pu = (proj_units(0) + proj_units(1)) if qc == 2 else []
                        if qc == 3:
                            drip = proj_units(2)
                        pb = {}
                        for bi, b in enumerate(bounds):
                            k = len(pu) // len(bounds)
                            pb[b] = pu[k * bi:k * (bi + 1)]
                        main2 = []
                        for i, u in enumerate(main):
                            main2.append(u)
                            for x in pb.get(i + 1, []):
                                main2.append(x)
                        _interleave(main2, drip)
                    for u in proj_units(NQ - 1):
                        u()
                elif 1 in phases:
                    for tc4 in range(NQ):
                        for u in p1_units(tc4):
                            u()

    if legalize:
        _legalize_waits(nc)
    return nc


def _host_tables():
    inv_freq = 1.0 / (10000.0 ** (np.arange(0, D, 2, dtype=np.float32) / D))
    t = np.arange(T, dtype=np.float32)
    freqs = np.outer(t, inv_freq).astype(np.float32)      # (T, 32)
    cos16 = np.cos(freqs).astype(np.float16)
    sin16 = np.sin(freqs).astype(np.float16)
    p = np.arange(128)[:, None]
    f = np.arange(128)[None, :]
    tri = (p <= f).astype(np.float16)                     # (128, 128)
    ztri = np.concatenate([np.zeros((128, 128), np.float16), tri], axis=1)
    return cos16, sin16, tri, ztri


_CACHE = {}


def kernel(x, v1, wq, wk, wv, wproj, lamb):
    x = np.asarray(x, dtype=np.float32)
    v1 = np.asarray(v1, dtype=np.float32)
    wq = np.asarray(wq, dtype=np.float32)
    wk = np.asarray(wk, dtype=np.float32)
    wv = np.asarray(wv, dtype=np.float32)
    wproj = np.asarray(wproj, dtype=np.float32)
    lam = float(np.asarray(lamb))

    cosn, sinn, tri, ztri = _host_tables()
    import ml_dtypes
    f8 = ml_dtypes.float8_e4m3

    def q3(a, scale):
        a = np.asarray(a * scale, dtype=np.float32)
        a8 = a.astype(f8)
        da8 = (a - a8.astype(np.float32)).astype(f8)
        return a8, da8

    key = lam
    if key not in _CACHE:
        _CACHE[key] = _build(lam)
    nc = _CACHE[key]

    in_maps = []
    for core in range(8):
        b, hg = core // 2, core % 2
        sl = slice(hg * HG, (hg + 1) * HG)
        x8, dx8 = q3(x[b].T, 1.0)
        wq8, dwq8 = q3(wq[sl, :].T, 64.0)
        wk8, dwk8 = q3(wk[sl, :].T, 64.0)
        wv8, dwv8 = q3(wv[sl, :].T, 64.0)
        in_maps.append({
            "x8T": np.ascontiguousarray(x8),
            "dx8T": np.ascontiguousarray(dx8),
            "v1b": np.ascontiguousarray(
                (lam * v1[b][:, sl]).astype(np.float16)),
            "wq8T": np.ascontiguousarray(wq8),
            "dwq8T": np.ascontiguousarray(dwq8),
            "wk8T": np.ascontiguousarray(wk8),
            "dwk8T": np.ascontiguousarray(dwk8),
            "wv8T": np.ascontiguousarray(wv8),
            "dwv8T": np.ascontiguousarray(dwv8),
            "wpT": np.ascontiguousarray(wproj[:, sl].T.astype(np.float16)),
            "cosn": cosn,
            "sinn": sinn,
            "tri01": tri,
            "ztri": ztri,
        })

    res = bass_utils.run_bass_kernel_spmd(nc, in_maps, core_ids=list(range(8)))
    y = np.empty((B, T, C), dtype=np.float32)
    for b in range(B):
        y[b] = (res.results[2 * b]["out"].astype(np.float32)
                + res.results[2 * b + 1]["out"].astype(np.float32))
    return (y, v1)


# revision 41
# speedup vs baseline: 1.0075x; 1.0026x over previous
"""Causal self-attention (B=4, T=2048, C=1024, H=16, D=64) on 8 TRN2 cores.

Sharding: core = 2*b + hg  (b = batch 0..3, hg = head-group 0..1 of 8 heads).
Each core computes its batch's QKV projections for its 8 heads (tensor
parallel over wq/wk/wv rows), RMSNorm+RoPE, causal attention, and a partial
output projection over its head-group's wproj columns. The two partials per
batch are summed on the host.

v7: fp8e4 DoubleRow matmuls with error-feedback for QKV and PV.
QKV runs three accumulating DoubleRow passes -- x8@w8 + dx8@w8 + x8@dw8 --
where dx8/dw8 are the fp8 quantization residuals (prepared on host, w
pre-scaled x64: the q/k scale cancels in RMSNorm, the v scale folds into the
value blend).  This cuts QKV PE time 25% below fp16 at ~0.1% extra error.
Values are stored as v8 + dv8 planes; PV runs two DoubleRow passes per
kt-pair (pt8 x v8, pt8 x dv8), contraction 256, with the ones-row riding in
v8 (zeros in dv8) for softmax denominators.  Probabilities go to fp8
directly from the ACT exp (bias -3 keeps exp inside fp8 range and cancels
in normalization).  Scores stay fp16 (qT/kT via DMA XBAR transpose);
RMSNorm applies AFTER RoPE (rotation commutes with the per-head scale).
Diagonal-strip exps cover both heads via one strided ACT instruction;
triangle masks use [tri]/[0|tri] tables on gpsimd (SBUF-only: TRN2 gpsimd
cannot access PSUM, and engine ops may read at most one PSUM operand --
hence the yu staging copy before the softmax normalization multiply, and
the PE broadcast matmul for 1/denominator).  fp8 matmul weight rows are
padded to 68 bytes (4-byte ISA alignment).

Emission is software-pipelined: engines execute their streams in order, so
QKV work for chunk qc+1 is interleaved into the attention pair-loop of
chunk qc at pair granularity, output-projection tiles for qc-1 drip at
head-pair boundaries (PSUM-ring-safe), and each head-pair's normalization
is delayed three pairs into the next head-pair so the recip/broadcast/mul
chain overlaps fresh scores/exp work.

The ISA has ONE semaphore-wait slot per instruction; Tile emits more.
_legalize_waits() splits extras onto same-engine NoOps post-scheduling.
"""

import math

import numpy as np

import concourse.bass as bass
import concourse.mybir as mybir
import concourse.tile as tile
from concourse import bass_utils

F32 = mybir.dt.float32
F16 = mybir.dt.float16
F8 = mybir.dt.float8e4
DR = mybir.MatmulPerfMode.DoubleRow

B, T, C, H, D = 4, 2048, 1024, 16, 64
HG = C // 2          # 512 features per head group (8 heads x 64)
NT = T // 128        # 16 t-tiles
NQ = T // 512        # 4 query/t chunks
EPS = 1.1920928955078125e-07
SCALE = 1.0 / math.sqrt(D)  # 0.125
EXP_BIAS = -3.0      # exp(s*SCALE + bias): keeps exp output < fp8e4 max

_wsplit_counter = [0]


def _legalize_waits(nc):
    """Split multi-wait instructions into single-wait NoOp chains."""
    n = 0
    for f in nc.m.functions:
        for bb in f.blocks:
            new_list = []
            changed = False
            for inst in bb.instructions:
                si = inst.sync_info
                if si is not None and si.on_wait and len(si.on_wait) > 1:
                    waits = list(si.on_wait)
                    for w in waits[:-1]:
                        _wsplit_counter[0] += 1
                        new_list.append(mybir.InstNoOp(
                            name=f"WSPLIT-{_wsplit_counter[0]}",
                            engine=inst.engine, ins=[], outs=[],
                            sync_info=mybir.SyncInfo(on_wait=[w], on_update=[]),
                        ))
                    si.on_wait = waits[-1:]
                    changed = True
                    n += 1
                new_list.append(inst)
            if changed:
                bb.instructions = new_list
    return n


def _interleave(main, extra):
    """Run main closures with extra closures dripped in proportionally."""
    n, m = len(main), len(extra)
    j = 0
    for i, u in enumerate(main):
        u()
        while m and j < m and j * n < m * (i + 1):
            extra[j]()
            j += 1
    while j < m:
        extra[j]()
        j += 1


def _build(lam: float, phases=(1, 2), legalize=True) -> bass.Bass:
    nc = bass.Bass("TRN2", target_bir_lowering=False, debug=False, num_devices=8)

    xb_d = nc.dram_tensor("x8T", [C, T], F8, kind="ExternalInput").ap()
    dxb_d = nc.dram_tensor("dx8T", [C, T], F8, kind="ExternalInput").ap()
    v1_d = nc.dram_tensor("v1b", [T, HG], F16, kind="ExternalInput").ap()
    wq_d = nc.dram_tensor("wq8T", [C, HG], F8, kind="ExternalInput").ap()
    dwq_d = nc.dram_tensor("dwq8T", [C, HG], F8, kind="ExternalInput").ap()
    wk_d = nc.dram_tensor("wk8T", [C, HG], F8, kind="ExternalInput").ap()
    dwk_d = nc.dram_tensor("dwk8T", [C, HG], F8, kind="ExternalInput").ap()
    wv_d = nc.dram_tensor("wv8T", [C, HG], F8, kind="ExternalInput").ap()
    dwv_d = nc.dram_tensor("dwv8T", [C, HG], F8, kind="ExternalInput").ap()
    wp_d = nc.dram_tensor("wpT", [HG, C], F16, kind="ExternalInput").ap()
    cos_d = nc.dram_tensor("cosn", [T, 32], F16, kind="ExternalInput").ap()
    sin_d = nc.dram_tensor("sinn", [T, 32], F16, kind="ExternalInput").ap()
    tri_d = nc.dram_tensor("tri01", [128, 128], F16, kind="ExternalInput").ap()
    ztri_d = nc.dram_tensor("ztri", [128, 256], F16, kind="ExternalInput").ap()
    out_d = nc.dram_tensor("out", [T, C], F16, kind="ExternalOutput").ap()

    with tile.TileContext(nc) as tc:
        with (
            tc.tile_pool(name="const", bufs=1) as const,
            tc.tile_pool(name="pers", bufs=1) as pers,
        ):
            epsc = const.tile([128, 1], F32)
            nc.vector.memset(epsc, EPS)
            ebias = const.tile([128, 1], F32)
            nc.vector.memset(ebias, EXP_BIAS)
            ones_f = const.tile([1, 64], F32)
            nc.vector.memset(ones_f, 1.0)
            ones1x64 = const.tile([1, 64], mybir.dt.float32r)
            nc.scalar.copy(out=ones1x64, in_=ones_f)

            # persistent transposed q/k (feature-major, f = 128*j + p)
            qT = pers.tile([128, 4, T], F16, name="qT", tag="qT")
            kT = pers.tile([128, 4, T], F16, name="kT", tag="kT")
            # values, kt-tile-paired for DoubleRow PV: [128k, plane, head, 65]
            # v8 = fp8(v); dv8 = fp8(v - v8).  Ones row: 1 in v8, 0 in dv8.
            vsp = [pers.tile([128, 2, 8, 68], F8, name=f"v{p}", tag=f"v{p}")
                   for p in range(NT // 2)]
            dvp = [pers.tile([128, 2, 8, 68], F8, name=f"dv{p}", tag=f"dv{p}")
                   for p in range(NT // 2)]
            for p in range(NT // 2):
                nc.vector.memset(vsp[p][:, :, :, 64:68], 0.0)
                nc.vector.memset(vsp[p][:, :, :, 64:65], 1.0)
                nc.gpsimd.memset(dvp[p][:, :, :, 64:68], 0.0)
            # fp8 probability ring (2 planes = one kt pair, 2 heads x 512 q)
            ptr = [pers.tile([128, 2, 1024], F8, name=f"pt{r}", tag=f"pt{r}")
                   for r in range(6)]
            for r in range(6):
                eng = nc.vector if r % 2 == 0 else nc.gpsimd
                eng.memset(ptr[r], 0.0)

            with (
                tc.tile_pool(name="p1", bufs=1) as p1,
                tc.tile_pool(name="p1ps", bufs=1, space="PSUM") as p1ps,
                tc.tile_pool(name="p2", bufs=1) as p2,
            ):
                # -------- startup: x chunk 0 first, then weights ---------
                xT_tiles = {}

                def load_x(tc4):
                    if tc4 >= NQ or tc4 in xT_tiles:
                        return
                    xt = p1.tile([128, 8, 512], F8, name="xT", tag="xT",
                                 bufs=4)
                    dxt = p1.tile([128, 8, 512], F8, name="dxT", tag="dxT",
                                  bufs=4)
                    t0 = tc4 * 512
                    for ts in range(4):
                        nc.sync.dma_start(
                            out=xt[:, :, ts * 128:(ts + 1) * 128],
                            in_=xb_d[:, t0 + ts * 128:t0 + (ts + 1) * 128]
                            .rearrange("(c p) t -> p c t", p=128))
                        nc.sync.dma_start(
                            out=dxt[:, :, ts * 128:(ts + 1) * 128],
                            in_=dxb_d[:, t0 + ts * 128:t0 + (ts + 1) * 128]
                            .rearrange("(c p) t -> p c t", p=128))
                    xT_tiles[tc4] = (xt, dxt)

                xt0 = p1.tile([128, 8, 512], F8, name="xT", tag="xT", bufs=4)
                dxt0 = p1.tile([128, 8, 512], F8, name="dxT", tag="dxT",
                               bufs=4)
                wq_sb = p1.tile([128, 2, 8, HG], F8)
                wk_sb = p1.tile([128, 2, 8, HG], F8)
                wv_sb = p1.tile([128, 2, 8, HG], F8)
                # x/dx slice 0 first (feeds the first Ldweights), then the wq
                # pair (first matmul), then remaining slices stream behind.
                nc.sync.dma_start(
                    out=xt0[:, :, 0:128],
                    in_=xb_d[:, 0:128].rearrange("(c p) t -> p c t", p=128))
                nc.sync.dma_start(
                    out=dxt0[:, :, 0:128],
                    in_=dxb_d[:, 0:128].rearrange("(c p) t -> p c t", p=128))
                nc.sync.dma_start(
                    out=wq_sb[:, 0, :, :],
                    in_=wq_d.rearrange("(c p) i -> p c i", p=128))
                nc.sync.dma_start(
                    out=wq_sb[:, 1, :, :],
                    in_=dwq_d.rearrange("(c p) i -> p c i", p=128))
                for ts in range(1, 4):
                    nc.sync.dma_start(
                        out=xt0[:, :, ts * 128:(ts + 1) * 128],
                        in_=xb_d[:, ts * 128:(ts + 1) * 128].rearrange(
                            "(c p) t -> p c t", p=128))
                    nc.sync.dma_start(
                        out=dxt0[:, :, ts * 128:(ts + 1) * 128],
                        in_=dxb_d[:, ts * 128:(ts + 1) * 128].rearrange(
                            "(c p) t -> p c t", p=128))
                xT_tiles[0] = (xt0, dxt0)
                nc.sync.dma_start(
                    out=wk_sb[:, 0, :, :],
                    in_=wk_d.rearrange("(c p) i -> p c i", p=128))
                nc.sync.dma_start(
                    out=wk_sb[:, 1, :, :],
                    in_=dwk_d.rearrange("(c p) i -> p c i", p=128))
                cos_sb = p1.tile([128, NT, 32], F16)
                nc.sync.dma_start(
                    out=cos_sb, in_=cos_d.rearrange("(n p) i -> p n i", p=128))
                sin_sb = p1.tile([128, NT, 32], F16)
                nc.sync.dma_start(
                    out=sin_sb, in_=sin_d.rearrange("(n p) i -> p n i", p=128))
                nc.sync.dma_start(
                    out=wv_sb[:, 0, :, :],
                    in_=wv_d.rearrange("(c p) i -> p c i", p=128))
                nc.sync.dma_start(
                    out=wv_sb[:, 1, :, :],
                    in_=dwv_d.rearrange("(c p) i -> p c i", p=128))
                tri01 = const.tile([128, 128], F16)
                nc.sync.dma_start(out=tri01, in_=tri_d)
                load_x(1)
                ztri = const.tile([128, 256], F16)
                nc.sync.dma_start(out=ztri, in_=ztri_d)

                w_sb = {"q": wq_sb, "k": wk_sb, "v": wv_sb}

                def p1_unit_a(tc4, which, ts):
                    """QKV matmuls + the op that releases the PSUM bank."""
                    x8t, dx8t = xT_tiles[tc4]
                    tg = tc4 * 4 + ts
                    ps = p1ps.tile([128, 512], F32, name="qkvps",
                                   tag="qkvps", bufs=2)
                    wpair = w_sb[which]
                    passes = ((x8t, 0), (dx8t, 0), (x8t, 1))
                    for pas, (xa, wi) in enumerate(passes):
                        for cc in range(4):
                            nc.tensor.matmul(
                                ps,
                                xa[:, 2 * cc:2 * cc + 2,
                                   ts * 128:(ts + 1) * 128],
                                wpair[:, wi, 2 * cc:2 * cc + 2, :],
                                start=(pas == 0 and cc == 0),
                                stop=(pas == 2 and cc == 3),
                                perf_mode=DR)
                    p3 = ps.rearrange("p (h d) -> p h d", h=8)
                    if which == "v":
                        v1t = p1.tile([128, HG], F16, name="v1t",
                                      tag="v1t", bufs=4)
                        nc.sync.dma_start(
                            out=v1t, in_=v1_d[tg * 128:(tg + 1) * 128, :])
                        vt16 = p1.tile([128, HG], F16, name="vt16",
                                       tag="vt16", bufs=2)
                        nc.vector.scalar_tensor_tensor(
                            out=vt16.rearrange("p (h d) -> p h d", h=8),
                            in0=p3, scalar=(1.0 - lam) / 64.0,
                            in1=v1t.rearrange("p (h d) -> p h d", h=8),
                            op0=mybir.AluOpType.mult,
                            op1=mybir.AluOpType.add)
                        return ("v", tg, vt16)

                    # fp16 copy releases the PSUM bank
                    qraw = p1.tile([128, 512], F16, name="qraw",
                                   tag="qraw", bufs=4)
                    nc.vector.tensor_copy(out=qraw, in_=ps)
                    return ("qk", tg, qraw, which)

                def p1_unit_b(tok):
                    if tok[0] == "v":
                        _, tg, vt16 = tok
                        v8dst = vsp[tg // 2][:, tg % 2, :, 0:64]
                        nc.gpsimd.tensor_copy(
                            out=v8dst,
                            in_=vt16.rearrange("p (h d) -> p h d", h=8))
                        nc.gpsimd.tensor_sub(
                            out=dvp[tg // 2][:, tg % 2, :, 0:64],
                            in0=vt16.rearrange("p (h d) -> p h d", h=8),
                            in1=v8dst)
                        return
                    _, tg, qraw, which = tok

                    # RMS stats
                    sqt = p1.tile([128, 512], F16, name="sqt",
                                  tag="sqt", bufs=2)
                    nc.vector.tensor_mul(out=sqt, in0=qraw, in1=qraw)
                    ssum = p1.tile([128, 8], F32, name="ssum",
                                   tag="ssum", bufs=4)
                    nc.vector.tensor_reduce(
                        ssum, sqt.rearrange("p (h d) -> p h d", h=8),
                        axis=mybir.AxisListType.X, op=mybir.AluOpType.add)
                    srt = p1.tile([128, 8], F32, name="srt", tag="srt",
                                  bufs=4)
                    nc.scalar.activation(
                        srt, ssum, mybir.ActivationFunctionType.Sqrt,
                        bias=epsc, scale=1.0 / 64.0)
                    rst = p1.tile([128, 8], F16, name="rst", tag="rst",
                                  bufs=4)
                    with nc.allow_low_precision(reason="rms scale in fp16"):
                        nc.vector.reciprocal(out=rst, in_=srt)

                    # RoPE on unnormalized q; per-head RMS scale applied last
                    rot = p1.tile([128, 512], F16, name="rot",
                                  tag="rot", bufs=2)
                    r3 = rot.rearrange("p (h d) -> p h d", h=8)
                    rc = p1.tile([128, 512], F16, name="rc", tag="rc", bufs=2)
                    rs = p1.tile([128, 512], F16, name="rs", tag="rs", bufs=2)
                    rc4 = rc.rearrange("p (h a i) -> p h a i", h=8, a=2)
                    rs4 = rs.rearrange("p (h a i) -> p h a i", h=8, a=2)
                    q4 = qraw.rearrange("p (h a i) -> p h a i", h=8, a=2)
                    c4 = cos_sb[:, tg, :].rearrange(
                        "p (o a i) -> p o a i", o=1, a=1).to_broadcast(
                        (128, 8, 2, 32))
                    s4 = sin_sb[:, tg, :].rearrange(
                        "p (o a i) -> p o a i", o=1, a=1).to_broadcast(
                        (128, 8, 2, 32))
                    nc.vector.tensor_mul(out=rc4, in0=q4, in1=c4)
                    nc.vector.tensor_mul(out=rs4, in0=q4, in1=s4)
                    # y1 = x1*c + x2*s ; y2 = x2*c - x1*s
                    nc.gpsimd.tensor_add(out=r3[:, :, 0:32],
                                         in0=rc4[:, :, 0, :],
                                         in1=rs4[:, :, 1, :])
                    nc.gpsimd.tensor_sub(out=r3[:, :, 32:64],
                                         in0=rc4[:, :, 1, :],
                                         in1=rs4[:, :, 0, :])
                    rstb = rst.rearrange("p (h o) -> p h o", o=1).to_broadcast(
                        (128, 8, 64))
                    rotn = p1.tile([128, 512], F16, name="rotn",
                                   tag="rotn", bufs=6)
                    nc.vector.tensor_mul(
                        out=rotn.rearrange("p (h d) -> p h d", h=8),
                        in0=r3, in1=rstb)

                    dstT = qT if which == "q" else kT
                    nc.sync.dma_start(
                        out=dstT[:, 0:4, tg * 128:(tg + 1) * 128],
                        in_=rotn, transpose=True)

                def p1_units(tc4):
                    units = [lambda t=tc4: load_x(t + 2)]
                    toks = []

                    def do_a(t, w, s, tk=toks):
                        tk.append(p1_unit_a(t, w, s))

                    def do_b(tk=toks):
                        if tk:
                            p1_unit_b(tk.pop(0))

                    seq = [(w, s) for w in ("q", "k", "v") for s in range(4)]

                    def ab(t, w, s):
                        do_a(t, w, s)
                        do_b()

                    for w, s in seq:
                        units.append(lambda t=tc4, w_=w, s_=s: ab(t, w_, s_))
                    return units

                yT_all = {}
                pair_ctr = [0]

                def p2_closures(qc):
                    """Returns (main_closures, hp_boundary_indices)."""
                    yT = [p2.tile([128, 512], F16, name=f"yT{qc}_{j}",
                                  tag=f"yT{qc}_{j}", bufs=1) for j in range(4)]
                    yT_all[qc] = yT

                    def make_hp(hp):
                        pair = (2 * hp, 2 * hp + 1)
                        npair = 2 * qc + 2
                        state = {"pv": None, "pending": []}

                        def alloc_pv():
                            state["pv"] = {h: p1ps.tile([68, 512], F32,
                                                        name="pv", tag="pv",
                                                        bufs=2)
                                           for h in pair}

                        def emit_pv(idx):
                            pt, pi, a0p = state["pending"][idx]
                            for h in pair:
                                o = 512 * (h % 2)
                                for pas, vt in ((0, vsp[pi]), (1, dvp[pi])):
                                    nc.tensor.matmul(
                                        state["pv"][h][:, a0p:512],
                                        vt[:, :, h, :],
                                        pt[:, :, o + a0p:o + 512],
                                        start=(pi == 0 and pas == 0),
                                        stop=(pi == npair - 1 and pas == 1),
                                        perf_mode=DR)

                        def do_pair(pidx):
                            if pidx == 0:
                                alloc_pv()
                            kt0, kt1 = 2 * pidx, 2 * pidx + 1
                            m0 = kt0 - 4 * qc
                            a0p = 128 * m0 if m0 >= 1 else 0
                            pt = ptr[pair_ctr[0] % 6]
                            pair_ctr[0] += 1
                            for j, kt in ((0, kt0), (1, kt1)):
                                m = kt - 4 * qc
                                a0s = 128 * m if m >= 1 else 0
                                st_t = p1ps.tile([128, 1024], F32, name="st",
                                                 tag="st", bufs=2)
                                for h in pair:
                                    b0 = 64 * (h % 2)
                                    o = 512 * (h % 2)
                                    nc.tensor.matmul(
                                        st_t[:, o + a0s:o + 512],
                                        kT[b0:b0 + 64, hp,
                                           kt * 128:(kt + 1) * 128],
                                        qT[b0:b0 + 64, hp,
                                           qc * 512 + a0s:(qc + 1) * 512],
                                        start=True, stop=True)
                                ptj = pt[:, j, :]
                                pt2 = ptj.rearrange("p (h q) -> p h q", h=2)
                                st2 = st_t.rearrange("p (h q) -> p h q", h=2)
                                if m <= 0:
                                    nc.scalar.activation(
                                        ptj, st_t,
                                        mybir.ActivationFunctionType.Exp,
                                        scale=SCALE, bias=ebias)
                                else:
                                    w0 = 128 * m
                                    nc.scalar.activation(
                                        pt2[:, :, w0:512], st2[:, :, w0:512],
                                        mybir.ActivationFunctionType.Exp,
                                        scale=SCALE, bias=ebias)
                                if m in (0, 2):
                                    nc.gpsimd.tensor_mul(
                                        out=pt2[:, :, a0s:a0s + 128],
                                        in0=pt2[:, :, a0s:a0s + 128],
                                        in1=tri01.rearrange(
                                            "p (o q) -> p o q",
                                            o=1).to_broadcast((128, 2, 128)))
                                elif m in (1, 3):
                                    nc.gpsimd.tensor_mul(
                                        out=pt2[:, :, a0p:a0p + 256],
                                        in0=pt2[:, :, a0p:a0p + 256],
                                        in1=ztri.rearrange(
                                            "p (o q) -> p o q",
                                            o=1).to_broadcast((128, 2, 256)))
                            state["pending"].append((pt, pidx, a0p))
                            if len(state["pending"]) >= 4:
                                emit_pv(len(state["pending"]) - 4)
                            if pidx == npair - 1:
                                for k in range(min(3, len(state["pending"])),
                                               0, -1):
                                    emit_pv(len(state["pending"]) - k)

                        def do_norm():
                            yus = {}
                            for h in pair:
                                yu = p2.tile([65, 512], F16, name="yu",
                                             tag="yu", bufs=4)
                                if h % 2 == 0:
                                    nc.scalar.copy(out=yu,
                                                   in_=state["pv"][h][0:65, :])
                                else:
                                    nc.vector.tensor_copy(
                                        out=yu, in_=state["pv"][h][0:65, :])
                                yus[h] = yu
                            for h in pair:
                                b0 = 64 * (h % 2)
                                yu = yus[h]
                                rec = p2.tile([1, 512], mybir.dt.float32r,
                                              name="rec", tag="rec", bufs=4)
                                with nc.allow_low_precision(
                                        reason="softmax denom recip"):
                                    nc.vector.reciprocal(
                                        out=rec, in_=yu[64:65, :])
                                rb = p1ps.tile([64, 512], F32, name="bcr",
                                               tag="pv", bufs=2)
                                nc.tensor.matmul(rb, ones1x64, rec,
                                                 start=True, stop=True)
                                nc.vector.tensor_mul(
                                    out=yT[hp][b0:b0 + 64, :],
                                    in0=yu[0:64, :], in1=rb)

                        units = [(lambda p=pidx: do_pair(p))
                                 for pidx in range(npair)]
                        units.append(do_norm)
                        return units

                    # Delay each hp's norm unit until after the next hp's
                    # first pair, so the recip->bcr->mul chain overlaps with
                    # fresh scores/exp work instead of stalling the engines
                    # at the head-pair boundary.
                    d = 3 if qc >= 1 else 2
                    main = []
                    bounds = []
                    held_norm = None
                    for hp in range(4):
                        units = make_hp(hp)
                        pairs_u, norm_u = units[:-1], units[-1]
                        main.extend(pairs_u[:d])
                        if held_norm is not None:
                            main.append(held_norm)
                            bounds.append(len(main))
                        main.extend(pairs_u[d:])
                        held_norm = norm_u
                    main.append(held_norm)
                    bounds.append(len(main))
                    return main, bounds

                wp_sb = p2.tile([128, 4, C], F16)

                def proj_unit(qc, tsub, jc):
                    yT = yT_all[qc]
                    pr = p1ps.tile([128, 512], F32, name="pr", tag="qkvps",
                                   bufs=2)
                    for ft in range(4):
                        nc.tensor.matmul(
                            pr,
                            yT[ft][:, tsub * 128:(tsub + 1) * 128],
                            wp_sb[:, ft, jc * 512:(jc + 1) * 512],
                            start=(ft == 0), stop=(ft == 3))
                    osb = p2.tile([128, 512], F16, name="osb", tag="osb",
                                  bufs=4)
                    nc.vector.tensor_copy(out=osb, in_=pr)
                    r0 = qc * 512 + tsub * 128
                    nc.sync.dma_start(
                        out=out_d[r0:r0 + 128, jc * 512:(jc + 1) * 512],
                        in_=osb)

                def proj_units(qc):
                    units = []
                    for tsub in range(4):
                        for jc in range(2):
                            units.append(
                                lambda q=qc, t=tsub, j=jc: proj_unit(q, t, j))
                    return units

                # ---------------- emission schedule ----------------
                if 1 in phases and 2 in phases:
                    for u in p1_units(0):
                        u()
                    nc.sync.dma_start(
                        out=wp_sb, in_=wp_d.rearrange("(c p) j -> p c j",
                                                      p=128))
                    for qc in range(NQ):
                        main, bounds = p2_closures(qc)
                        drip = p1_units(qc + 1) if qc + 1 < NQ else []
                        # proj(0)+proj(1) at p2(2) hp boundaries; proj(2) is
                        # fine-dripped through p2(3) (qkvps ring is idle
                        # there); proj(3) runs at the tail.
                        pu = (proj_units(0) + proj_units(1)) if qc == 2 else []
                        if qc == 3:
                            drip = proj_units(2)
                        pb = {}
                        for bi, b in enumerate(bounds):
                            k = len(pu) // len(bounds)
                            pb[b] = pu[k * bi:k * (bi + 1)]
                        main2 = []
                        for i, u in enumerate(main):
                            main2.append(u)
                            for x in pb.get(i + 1, []):
                                main2.append(x)
                        _interleave(main2, drip)
                    for u in proj_units(NQ - 1):
                        u()
                elif 1 in phases:
                    for tc4 in range(NQ):
                        for u in p1_units(tc4):
                            u()

    if legalize:
        _legalize_waits(nc)
    return nc


def _host_tables():
    inv_freq = 1.0 / (10000.0 ** (np.arange(0, D, 2, dtype=np.float32) / D))
    t = np.arange(T, dtype=np.float32)
    freqs = np.outer(t, inv_freq).astype(np.float32)      # (T, 32)
    cos16 = np.cos(freqs).astype(np.float16)
    sin16 = np.sin(freqs).astype(np.float16)
    p = np.arange(128)[:, None]
    f = np.arange(128)[None, :]
    tri = (p <= f).astype(np.float16)                     # (128, 128)
    ztri = np.concatenate([np.zeros((128, 128), np.float16), tri], axis=1)
    return cos16, sin16, tri, ztri


_CACHE = {}


def kernel(x, v1, wq, wk, wv, wproj, lamb):
    x = np.asarray(x, dtype=np.float32)
    v1 = np.asarray(v1, dtype=np.float32)
    wq = np.asarray(wq, dtype=np.float32)
    wk = np.asarray(wk, dtype=np.float32)
    wv = np.asarray(wv, dtype=np.float32)
    wproj = np.asarray(wproj, dtype=np.float32)
    lam = float(np.asarray(lamb))

    cosn, sinn, tri, ztri = _host_tables()
    import ml_dtypes
    f8 = ml_dtypes.float8_e4m3

    def q3(a, scale):
        a = np.asarray(a * scale, dtype=np.float32)
        a8 = a.astype(f8)
        da8 = (a - a8.astype(np.float32)).astype(f8)
        return a8, da8

    key = lam
    if key not in _CACHE:
        _CACHE[key] = _build(lam)
    nc = _CACHE[key]

    in_maps = []
    for core in range(8):
        b, hg = core // 2, core % 2
        sl = slice(hg * HG, (hg + 1) * HG)
        x8, dx8 = q3(x[b].T, 1.0)
        wq8, dwq8 = q3(wq[sl, :].T, 64.0)
        wk8, dwk8 = q3(wk[sl, :].T, 64.0)
        wv8, dwv8 = q3(wv[sl, :].T, 64.0)
        in_maps.append({
            "x8T": np.ascontiguousarray(x8),
            "dx8T": np.ascontiguousarray(dx8),
            "v1b": np.ascontiguousarray(
                (lam * v1[b][:, sl]).astype(np.float16)),
            "wq8T": np.ascontiguousarray(wq8),
            "dwq8T": np.ascontiguousarray(dwq8),
            "wk8T": np.ascontiguousarray(wk8),
            "dwk8T": np.ascontiguousarray(dwk8),
            "wv8T": np.ascontiguousarray(wv8),
            "dwv8T": np.ascontiguousarray(dwv8),
            "wpT": np.ascontiguousarray(wproj[:, sl].T.astype(np.float16)),
            "cosn": cosn,
            "sinn": sinn,
            "tri01": tri,
            "ztri": ztri,
        })

    res = bass_utils.run_bass_kernel_spmd(nc, in_maps, core_ids=list(range(8)))
    y = np.empty((B, T, C), dtype=np.float32)
    for b in range(B):
        y[b] = (res.results[2 * b]["out"].astype(np.float32)
                + res.results[2 * b + 1]["out"].astype(np.float32))
    return (y, v1)


# revision 42
# speedup vs baseline: 1.0115x; 1.0040x over previous
"""Causal self-attention (B=4, T=2048, C=1024, H=16, D=64) on 8 TRN2 cores.

Sharding: core = 2*b + hg  (b = batch 0..3, hg = head-group 0..1 of 8 heads).
Each core computes its batch's QKV projections for its 8 heads (tensor
parallel over wq/wk/wv rows), RMSNorm+RoPE, causal attention, and a partial
output projection over its head-group's wproj columns. The two partials per
batch are summed on the host.

v7: fp8e4 DoubleRow matmuls with error-feedback for QKV and PV.
QKV runs three accumulating DoubleRow passes -- x8@w8 + dx8@w8 + x8@dw8 --
where dx8/dw8 are the fp8 quantization residuals (prepared on host, w
pre-scaled x64: the q/k scale cancels in RMSNorm, the v scale folds into the
value blend).  This cuts QKV PE time 25% below fp16 at ~0.1% extra error.
Values are stored as v8 + dv8 planes; PV runs two DoubleRow passes per
kt-pair (pt8 x v8, pt8 x dv8), contraction 256, with the ones-row riding in
v8 (zeros in dv8) for softmax denominators.  Probabilities go to fp8
directly from the ACT exp (bias -3 keeps exp inside fp8 range and cancels
in normalization).  Scores stay fp16 (qT/kT via DMA XBAR transpose);
RMSNorm applies AFTER RoPE (rotation commutes with the per-head scale).
Diagonal-strip exps cover both heads via one strided ACT instruction;
triangle masks use [tri]/[0|tri] tables on gpsimd (SBUF-only: TRN2 gpsimd
cannot access PSUM, and engine ops may read at most one PSUM operand --
hence the yu staging copy before the softmax normalization multiply, and
the PE broadcast matmul for 1/denominator).  fp8 matmul weight rows are
padded to 68 bytes (4-byte ISA alignment).

Emission is software-pipelined: engines execute their streams in order, so
QKV work for chunk qc+1 is interleaved into the attention pair-loop of
chunk qc at pair granularity; output-projection tiles for chunks 0+1 drip
at p2(2) head-pair boundaries and chunk 2's are fine-dripped through the
ACT-bound p2(3) phase (where the qkvps PSUM ring is otherwise idle); each
head-pair's normalization is delayed three pairs into the next head-pair
(bcr rides the pv PSUM ring, keeping the st ring free for scores) so the
recip/broadcast/mul chain overlaps fresh scores/exp work.  x chunks are
prefetched two phases ahead (ring of 4) with the first t-slice landing
before the weights so the opening Ldweights starts immediately.

The ISA has ONE semaphore-wait slot per instruction; Tile emits more.
_legalize_waits() splits extras onto same-engine NoOps post-scheduling.
"""

import math

import numpy as np

import concourse.bass as bass
import concourse.mybir as mybir
import concourse.tile as tile
from concourse import bass_utils

F32 = mybir.dt.float32
F16 = mybir.dt.float16
F8 = mybir.dt.float8e4
DR = mybir.MatmulPerfMode.DoubleRow

B, T, C, H, D = 4, 2048, 1024, 16, 64
HG = C // 2          # 512 features per head group (8 heads x 64)
NT = T // 128        # 16 t-tiles
NQ = T // 512        # 4 query/t chunks
EPS = 1.1920928955078125e-07
SCALE = 1.0 / math.sqrt(D)  # 0.125
EXP_BIAS = -3.0      # exp(s*SCALE + bias): keeps exp output < fp8e4 max

_wsplit_counter = [0]


def _legalize_waits(nc):
    """Split multi-wait instructions into single-wait NoOp chains."""
    n = 0
    for f in nc.m.functions:
        for bb in f.blocks:
            new_list = []
            changed = False
            for inst in bb.instructions:
                si = inst.sync_info
                if si is not None and si.on_wait and len(si.on_wait) > 1:
                    waits = list(si.on_wait)
                    for w in waits[:-1]:
                        _wsplit_counter[0] += 1
                        new_list.append(mybir.InstNoOp(
                            name=f"WSPLIT-{_wsplit_counter[0]}",
                            engine=inst.engine, ins=[], outs=[],
                            sync_info=mybir.SyncInfo(on_wait=[w], on_update=[]),
                        ))
                    si.on_wait = waits[-1:]
                    changed = True
                    n += 1
                new_list.append(inst)
            if changed:
                bb.instructions = new_list
    return n


def _interleave(main, extra):
    """Run main closures with extra closures dripped in proportionally."""
    n, m = len(main), len(extra)
    j = 0
    for i, u in enumerate(main):
        u()
        while m and j < m and j * n < m * (i + 1):
            extra[j]()
            j += 1
    while j < m:
        extra[j]()
        j += 1


def _build(lam: float, phases=(1, 2), legalize=True) -> bass.Bass:
    nc = bass.Bass("TRN2", target_bir_lowering=False, debug=False, num_devices=8)

    xb_d = nc.dram_tensor("x8T", [C, T], F8, kind="ExternalInput").ap()
    dxb_d = nc.dram_tensor("dx8T", [C, T], F8, kind="ExternalInput").ap()
    v1_d = nc.dram_tensor("v1b", [T, HG], F16, kind="ExternalInput").ap()
    wq_d = nc.dram_tensor("wq8T", [C, HG], F8, kind="ExternalInput").ap()
    dwq_d = nc.dram_tensor("dwq8T", [C, HG], F8, kind="ExternalInput").ap()
    wk_d = nc.dram_tensor("wk8T", [C, HG], F8, kind="ExternalInput").ap()
    dwk_d = nc.dram_tensor("dwk8T", [C, HG], F8, kind="ExternalInput").ap()
    wv_d = nc.dram_tensor("wv8T", [C, HG], F8, kind="ExternalInput").ap()
    dwv_d = nc.dram_tensor("dwv8T", [C, HG], F8, kind="ExternalInput").ap()
    wp_d = nc.dram_tensor("wpT", [HG, C], F16, kind="ExternalInput").ap()
    cos_d = nc.dram_tensor("cosn", [T, 32], F16, kind="ExternalInput").ap()
    sin_d = nc.dram_tensor("sinn", [T, 32], F16, kind="ExternalInput").ap()
    tri_d = nc.dram_tensor("tri01", [128, 128], F16, kind="ExternalInput").ap()
    ztri_d = nc.dram_tensor("ztri", [128, 256], F16, kind="ExternalInput").ap()
    out_d = nc.dram_tensor("out", [T, C], F16, kind="ExternalOutput").ap()

    with tile.TileContext(nc) as tc:
        with (
            tc.tile_pool(name="const", bufs=1) as const,
            tc.tile_pool(name="pers", bufs=1) as pers,
        ):
            epsc = const.tile([128, 1], F32)
            nc.vector.memset(epsc, EPS)
            ebias = const.tile([128, 1], F32)
            nc.vector.memset(ebias, EXP_BIAS)
            ones_f = const.tile([1, 64], F32)
            nc.vector.memset(ones_f, 1.0)
            ones1x64 = const.tile([1, 64], mybir.dt.float32r)
            nc.scalar.copy(out=ones1x64, in_=ones_f)

            # persistent transposed q/k (feature-major, f = 128*j + p)
            qT = pers.tile([128, 4, T], F16, name="qT", tag="qT")
            kT = pers.tile([128, 4, T], F16, name="kT", tag="kT")
            # values, kt-tile-paired for DoubleRow PV: [128k, plane, head, 65]
            # v8 = fp8(v); dv8 = fp8(v - v8).  Ones row: 1 in v8, 0 in dv8.
            vsp = [pers.tile([128, 2, 8, 68], F8, name=f"v{p}", tag=f"v{p}")
                   for p in range(NT // 2)]
            dvp = [pers.tile([128, 2, 8, 68], F8, name=f"dv{p}", tag=f"dv{p}")
                   for p in range(NT // 2)]
            for p in range(NT // 2):
                nc.vector.memset(vsp[p][:, :, :, 64:68], 0.0)
                nc.vector.memset(vsp[p][:, :, :, 64:65], 1.0)
                nc.gpsimd.memset(dvp[p][:, :, :, 64:68], 0.0)
            # fp8 probability ring (2 planes = one kt pair, 2 heads x 512 q)
            ptr = [pers.tile([128, 2, 1024], F8, name=f"pt{r}", tag=f"pt{r}")
                   for r in range(6)]
            for r in range(6):
                eng = nc.vector if r % 2 == 0 else nc.gpsimd
                eng.memset(ptr[r], 0.0)

            with (
                tc.tile_pool(name="p1", bufs=1) as p1,
                tc.tile_pool(name="p1ps", bufs=1, space="PSUM") as p1ps,
                tc.tile_pool(name="p2", bufs=1) as p2,
            ):
                # -------- startup: x chunk 0 first, then weights ---------
                xT_tiles = {}

                def load_x(tc4):
                    if tc4 >= NQ or tc4 in xT_tiles:
                        return
                    xt = p1.tile([128, 8, 512], F8, name="xT", tag="xT",
                                 bufs=4)
                    dxt = p1.tile([128, 8, 512], F8, name="dxT", tag="dxT",
                                  bufs=4)
                    t0 = tc4 * 512
                    for ts in range(4):
                        nc.sync.dma_start(
                            out=xt[:, :, ts * 128:(ts + 1) * 128],
                            in_=xb_d[:, t0 + ts * 128:t0 + (ts + 1) * 128]
                            .rearrange("(c p) t -> p c t", p=128))
                        nc.sync.dma_start(
                            out=dxt[:, :, ts * 128:(ts + 1) * 128],
                            in_=dxb_d[:, t0 + ts * 128:t0 + (ts + 1) * 128]
                            .rearrange("(c p) t -> p c t", p=128))
                    xT_tiles[tc4] = (xt, dxt)

                xt0 = p1.tile([128, 8, 512], F8, name="xT", tag="xT", bufs=4)
                dxt0 = p1.tile([128, 8, 512], F8, name="dxT", tag="dxT",
                               bufs=4)
                wq_sb = p1.tile([128, 2, 8, HG], F8)
                wk_sb = p1.tile([128, 2, 8, HG], F8)
                wv_sb = p1.tile([128, 2, 8, HG], F8)
                # x/dx slice 0 first (feeds the first Ldweights), then the wq
                # pair (first matmul), then remaining slices stream behind.
                nc.sync.dma_start(
                    out=xt0[:, :, 0:128],
                    in_=xb_d[:, 0:128].rearrange("(c p) t -> p c t", p=128))
                nc.sync.dma_start(
                    out=dxt0[:, :, 0:128],
                    in_=dxb_d[:, 0:128].rearrange("(c p) t -> p c t", p=128))
                nc.sync.dma_start(
                    out=wq_sb[:, 0, :, :],
                    in_=wq_d.rearrange("(c p) i -> p c i", p=128))
                nc.sync.dma_start(
                    out=wq_sb[:, 1, :, :],
                    in_=dwq_d.rearrange("(c p) i -> p c i", p=128))
                for ts in range(1, 4):
                    nc.sync.dma_start(
                        out=xt0[:, :, ts * 128:(ts + 1) * 128],
                        in_=xb_d[:, ts * 128:(ts + 1) * 128].rearrange(
                            "(c p) t -> p c t", p=128))
                    nc.sync.dma_start(
                        out=dxt0[:, :, ts * 128:(ts + 1) * 128],
                        in_=dxb_d[:, ts * 128:(ts + 1) * 128].rearrange(
                            "(c p) t -> p c t", p=128))
                xT_tiles[0] = (xt0, dxt0)
                nc.sync.dma_start(
                    out=wk_sb[:, 0, :, :],
                    in_=wk_d.rearrange("(c p) i -> p c i", p=128))
                nc.sync.dma_start(
                    out=wk_sb[:, 1, :, :],
                    in_=dwk_d.rearrange("(c p) i -> p c i", p=128))
                cos_sb = p1.tile([128, NT, 32], F16)
                nc.sync.dma_start(
                    out=cos_sb, in_=cos_d.rearrange("(n p) i -> p n i", p=128))
                sin_sb = p1.tile([128, NT, 32], F16)
                nc.sync.dma_start(
                    out=sin_sb, in_=sin_d.rearrange("(n p) i -> p n i", p=128))
                nc.sync.dma_start(
                    out=wv_sb[:, 0, :, :],
                    in_=wv_d.rearrange("(c p) i -> p c i", p=128))
                nc.sync.dma_start(
                    out=wv_sb[:, 1, :, :],
                    in_=dwv_d.rearrange("(c p) i -> p c i", p=128))
                tri01 = const.tile([128, 128], F16)
                nc.sync.dma_start(out=tri01, in_=tri_d)
                load_x(1)
                ztri = const.tile([128, 256], F16)
                nc.sync.dma_start(out=ztri, in_=ztri_d)

                w_sb = {"q": wq_sb, "k": wk_sb, "v": wv_sb}

                def p1_unit_a(tc4, which, ts):
                    """QKV matmuls + the op that releases the PSUM bank."""
                    x8t, dx8t = xT_tiles[tc4]
                    tg = tc4 * 4 + ts
                    ps = p1ps.tile([128, 512], F32, name="qkvps",
                                   tag="qkvps", bufs=2)
                    wpair = w_sb[which]
                    passes = ((x8t, 0), (dx8t, 0), (x8t, 1))
                    for pas, (xa, wi) in enumerate(passes):
                        for cc in range(4):
                            nc.tensor.matmul(
                                ps,
                                xa[:, 2 * cc:2 * cc + 2,
                                   ts * 128:(ts + 1) * 128],
                                wpair[:, wi, 2 * cc:2 * cc + 2, :],
                                start=(pas == 0 and cc == 0),
                                stop=(pas == 2 and cc == 3),
                                perf_mode=DR)
                    p3 = ps.rearrange("p (h d) -> p h d", h=8)
                    if which == "v":
                        v1t = p1.tile([128, HG], F16, name="v1t",
                                      tag="v1t", bufs=4)
                        nc.sync.dma_start(
                            out=v1t, in_=v1_d[tg * 128:(tg + 1) * 128, :])
                        vt16 = p1.tile([128, HG], F16, name="vt16",
                                       tag="vt16", bufs=2)
                        nc.vector.scalar_tensor_tensor(
                            out=vt16.rearrange("p (h d) -> p h d", h=8),
                            in0=p3, scalar=(1.0 - lam) / 64.0,
                            in1=v1t.rearrange("p (h d) -> p h d", h=8),
                            op0=mybir.AluOpType.mult,
                            op1=mybir.AluOpType.add)
                        return ("v", tg, vt16)

                    # fp16 copy releases the PSUM bank
                    qraw = p1.tile([128, 512], F16, name="qraw",
                                   tag="qraw", bufs=4)
                    nc.vector.tensor_copy(out=qraw, in_=ps)
                    return ("qk", tg, qraw, which)

                def p1_unit_b(tok):
                    if tok[0] == "v":
                        _, tg, vt16 = tok
                        v8dst = vsp[tg // 2][:, tg % 2, :, 0:64]
                        nc.gpsimd.tensor_copy(
                            out=v8dst,
                            in_=vt16.rearrange("p (h d) -> p h d", h=8))
                        nc.gpsimd.tensor_sub(
                            out=dvp[tg // 2][:, tg % 2, :, 0:64],
                            in0=vt16.rearrange("p (h d) -> p h d", h=8),
                            in1=v8dst)
                        return
                    _, tg, qraw, which = tok

                    # RMS stats
                    sqt = p1.tile([128, 512], F16, name="sqt",
                                  tag="sqt", bufs=2)
                    nc.vector.tensor_mul(out=sqt, in0=qraw, in1=qraw)
                    ssum = p1.tile([128, 8], F32, name="ssum",
                                   tag="ssum", bufs=4)
                    nc.vector.tensor_reduce(
                        ssum, sqt.rearrange("p (h d) -> p h d", h=8),
                        axis=mybir.AxisListType.X, op=mybir.AluOpType.add)
                    srt = p1.tile([128, 8], F32, name="srt", tag="srt",
                                  bufs=4)
                    nc.scalar.activation(
                        srt, ssum, mybir.ActivationFunctionType.Sqrt,
                        bias=epsc, scale=1.0 / 64.0)
                    rst = p1.tile([128, 8], F16, name="rst", tag="rst",
                                  bufs=4)
                    with nc.allow_low_precision(reason="rms scale in fp16"):
                        nc.vector.reciprocal(out=rst, in_=srt)

                    # RoPE on unnormalized q; per-head RMS scale applied last
                    rot = p1.tile([128, 512], F16, name="rot",
                                  tag="rot", bufs=2)
                    r3 = rot.rearrange("p (h d) -> p h d", h=8)
                    rc = p1.tile([128, 512], F16, name="rc", tag="rc", bufs=2)
                    rs = p1.tile([128, 512], F16, name="rs", tag="rs", bufs=2)
                    rc4 = rc.rearrange("p (h a i) -> p h a i", h=8, a=2)
                    rs4 = rs.rearrange("p (h a i) -> p h a i", h=8, a=2)
                    q4 = qraw.rearrange("p (h a i) -> p h a i", h=8, a=2)
                    c4 = cos_sb[:, tg, :].rearrange(
                        "p (o a i) -> p o a i", o=1, a=1).to_broadcast(
                        (128, 8, 2, 32))
                    s4 = sin_sb[:, tg, :].rearrange(
                        "p (o a i) -> p o a i", o=1, a=1).to_broadcast(
                        (128, 8, 2, 32))
                    nc.vector.tensor_mul(out=rc4, in0=q4, in1=c4)
                    nc.vector.tensor_mul(out=rs4, in0=q4, in1=s4)
                    # y1 = x1*c + x2*s ; y2 = x2*c - x1*s
                    nc.gpsimd.tensor_add(out=r3[:, :, 0:32],
                                         in0=rc4[:, :, 0, :],
                                         in1=rs4[:, :, 1, :])
                    nc.gpsimd.tensor_sub(out=r3[:, :, 32:64],
                                         in0=rc4[:, :, 1, :],
                                         in1=rs4[:, :, 0, :])
                    rstb = rst.rearrange("p (h o) -> p h o", o=1).to_broadcast(
                        (128, 8, 64))
                    rotn = p1.tile([128, 512], F16, name="rotn",
                                   tag="rotn", bufs=6)
                    nc.vector.tensor_mul(
                        out=rotn.rearrange("p (h d) -> p h d", h=8),
                        in0=r3, in1=rstb)

                    dstT = qT if which == "q" else kT
                    nc.sync.dma_start(
                        out=dstT[:, 0:4, tg * 128:(tg + 1) * 128],
                        in_=rotn, transpose=True)

                def p1_units(tc4):
                    units = [lambda t=tc4: load_x(t + 2)]
                    toks = []

                    def do_a(t, w, s, tk=toks):
                        tk.append(p1_unit_a(t, w, s))

                    def do_b(tk=toks):
                        if tk:
                            p1_unit_b(tk.pop(0))

                    seq = [(w, s) for w in ("q", "k", "v") for s in range(4)]

                    def ab(t, w, s):
                        do_a(t, w, s)
                        do_b()

                    for w, s in seq:
                        units.append(lambda t=tc4, w_=w, s_=s: ab(t, w_, s_))
                    return units

                yT_all = {}
                pair_ctr = [0]

                def p2_closures(qc):
                    """Returns (main_closures, hp_boundary_indices)."""
                    yT = [p2.tile([128, 512], F16, name=f"yT{qc}_{j}",
                                  tag=f"yT{qc}_{j}", bufs=1) for j in range(4)]
                    yT_all[qc] = yT

                    def make_hp(hp):
                        pair = (2 * hp, 2 * hp + 1)
                        npair = 2 * qc + 2
                        state = {"pv": None, "pending": []}

                        def alloc_pv():
                            state["pv"] = {h: p1ps.tile([68, 512], F32,
                                                        name="pv", tag="pv",
                                                        bufs=2)
                                           for h in pair}

                        def emit_pv(idx):
                            pt, pi, a0p = state["pending"][idx]
                            for h in pair:
                                o = 512 * (h % 2)
                                for pas, vt in ((0, vsp[pi]), (1, dvp[pi])):
                                    nc.tensor.matmul(
                                        state["pv"][h][:, a0p:512],
                                        vt[:, :, h, :],
                                        pt[:, :, o + a0p:o + 512],
                                        start=(pi == 0 and pas == 0),
                                        stop=(pi == npair - 1 and pas == 1),
                                        perf_mode=DR)

                        def do_pair(pidx):
                            if pidx == 0:
                                alloc_pv()
                            kt0, kt1 = 2 * pidx, 2 * pidx + 1
                            m0 = kt0 - 4 * qc
                            a0p = 128 * m0 if m0 >= 1 else 0
                            pt = ptr[pair_ctr[0] % 6]
                            pair_ctr[0] += 1
                            for j, kt in ((0, kt0), (1, kt1)):
                                m = kt - 4 * qc
                                a0s = 128 * m if m >= 1 else 0
                                st_t = p1ps.tile([128, 1024], F32, name="st",
                                                 tag="st", bufs=2)
                                for h in pair:
                                    b0 = 64 * (h % 2)
                                    o = 512 * (h % 2)
                                    nc.tensor.matmul(
                                        st_t[:, o + a0s:o + 512],
                                        kT[b0:b0 + 64, hp,
                                           kt * 128:(kt + 1) * 128],
                                        qT[b0:b0 + 64, hp,
                                           qc * 512 + a0s:(qc + 1) * 512],
                                        start=True, stop=True)
                                ptj = pt[:, j, :]
                                pt2 = ptj.rearrange("p (h q) -> p h q", h=2)
                                st2 = st_t.rearrange("p (h q) -> p h q", h=2)
                                if m <= 0:
                                    nc.scalar.activation(
                                        ptj, st_t,
                                        mybir.ActivationFunctionType.Exp,
                                        scale=SCALE, bias=ebias)
                                else:
                                    w0 = 128 * m
                                    nc.scalar.activation(
                                        pt2[:, :, w0:512], st2[:, :, w0:512],
                                        mybir.ActivationFunctionType.Exp,
                                        scale=SCALE, bias=ebias)
                                if m in (0, 2):
                                    nc.gpsimd.tensor_mul(
                                        out=pt2[:, :, a0s:a0s + 128],
                                        in0=pt2[:, :, a0s:a0s + 128],
                                        in1=tri01.rearrange(
                                            "p (o q) -> p o q",
                                            o=1).to_broadcast((128, 2, 128)))
                                elif m in (1, 3):
                                    nc.gpsimd.tensor_mul(
                                        out=pt2[:, :, a0p:a0p + 256],
                                        in0=pt2[:, :, a0p:a0p + 256],
                                        in1=ztri.rearrange(
                                            "p (o q) -> p o q",
                                            o=1).to_broadcast((128, 2, 256)))
                            state["pending"].append((pt, pidx, a0p))
                            if len(state["pending"]) >= 4:
                                emit_pv(len(state["pending"]) - 4)
                            if pidx == npair - 1:
                                for k in range(min(3, len(state["pending"])),
                                               0, -1):
                                    emit_pv(len(state["pending"]) - k)

                        def do_norm():
                            yus = {}
                            for h in pair:
                                yu = p2.tile([65, 512], F16, name="yu",
                                             tag="yu", bufs=4)
                                if h % 2 == 0:
                                    nc.scalar.copy(out=yu,
                                                   in_=state["pv"][h][0:65, :])
                                else:
                                    nc.vector.tensor_copy(
                                        out=yu, in_=state["pv"][h][0:65, :])
                                yus[h] = yu
                            for h in pair:
                                b0 = 64 * (h % 2)
                                yu = yus[h]
                                rec = p2.tile([1, 512], mybir.dt.float32r,
                                              name="rec", tag="rec", bufs=4)
                                with nc.allow_low_precision(
                                        reason="softmax denom recip"):
                                    nc.vector.reciprocal(
                                        out=rec, in_=yu[64:65, :])
                                rb = p1ps.tile([64, 512], F32, name="bcr",
                                               tag="pv", bufs=2)
                                nc.tensor.matmul(rb, ones1x64, rec,
                                                 start=True, stop=True)
                                nc.vector.tensor_mul(
                                    out=yT[hp][b0:b0 + 64, :],
                                    in0=yu[0:64, :], in1=rb)

                        units = [(lambda p=pidx: do_pair(p))
                                 for pidx in range(npair)]
                        units.append(do_norm)
                        return units

                    # Delay each hp's norm unit until after the next hp's
                    # first pair, so the recip->bcr->mul chain overlaps with
                    # fresh scores/exp work instead of stalling the engines
                    # at the head-pair boundary.
                    d = 3 if qc >= 1 else 2
                    main = []
                    bounds = []
                    held_norm = None
                    for hp in range(4):
                        units = make_hp(hp)
                        pairs_u, norm_u = units[:-1], units[-1]
                        main.extend(pairs_u[:d])
                        if held_norm is not None:
                            main.append(held_norm)
                            bounds.append(len(main))
                        main.extend(pairs_u[d:])
                        held_norm = norm_u
                    main.append(held_norm)
                    bounds.append(len(main))
                    return main, bounds

                wp_sb = p2.tile([128, 4, C], F16)

                def proj_unit(qc, tsub, jc):
                    yT = yT_all[qc]
                    pr = p1ps.tile([128, 512], F32, name="pr", tag="qkvps",
                                   bufs=2)
                    for ft in range(4):
                        nc.tensor.matmul(
                            pr,
                            yT[ft][:, tsub * 128:(tsub + 1) * 128],
                            wp_sb[:, ft, jc * 512:(jc + 1) * 512],
                            start=(ft == 0), stop=(ft == 3))
                    osb = p2.tile([128, 512], F16, name="osb", tag="osb",
                                  bufs=4)
                    nc.vector.tensor_copy(out=osb, in_=pr)
                    r0 = qc * 512 + tsub * 128
                    nc.sync.dma_start(
                        out=out_d[r0:r0 + 128, jc * 512:(jc + 1) * 512],
                        in_=osb)

                def proj_units(qc):
                    units = []
                    for tsub in range(4):
                        for jc in range(2):
                            units.append(
                                lambda q=qc, t=tsub, j=jc: proj_unit(q, t, j))
                    return units

                # ---------------- emission schedule ----------------
                if 1 in phases and 2 in phases:
                    for u in p1_units(0):
                        u()
                    nc.sync.dma_start(
                        out=wp_sb, in_=wp_d.rearrange("(c p) j -> p c j",
                                                      p=128))
                    for qc in range(NQ):
                        main, bounds = p2_closures(qc)
                        drip = p1_units(qc + 1) if qc + 1 < NQ else []
                        # proj(0)+proj(1) at p2(2) hp boundaries; proj(2) is
                        # fine-dripped through p2(3) (qkvps ring is idle
                        # there); proj(3) runs at the tail.
                        pu = (proj_units(0) + proj_units(1)) if qc == 2 else []
                        if qc == 3:
                            drip = proj_units(2)
                        pb = {}
                        for bi, b in enumerate(bounds):
                            k = len(pu) // len(bounds)
                            pb[b] = pu[k * bi:k * (bi + 1)]
                        main2 = []
                        for i, u in enumerate(main):
                            main2.append(u)
                            for x in pb.get(i + 1, []):
                                main2.append(x)
                        _interleave(main2, drip)
                    for u in proj_units(NQ - 1):
                        u()
                elif 1 in phases:
                    for tc4 in range(NQ):
                        for u in p1_units(tc4):
                            u()

    if legalize:
        _legalize_waits(nc)
    return nc


def _host_tables():
    inv_freq = 1.0 / (10000.0 ** (np.arange(0, D, 2, dtype=np.float32) / D))
    t = np.arange(T, dtype=np.float32)
    freqs = np.outer(t, inv_freq).astype(np.float32)      # (T, 32)
    cos16 = np.cos(freqs).astype(np.float16)
    sin16 = np.sin(freqs).astype(np.float16)
    p = np.arange(128)[:, None]
    f = np.arange(128)[None, :]
    tri = (p <= f).astype(np.float16)                     # (128, 128)
    ztri = np.concatenate([np.zeros((128, 128), np.float16), tri], axis=1)
    return cos16, sin16, tri, ztri


_CACHE = {}


def kernel(x, v1, wq, wk, wv, wproj, lamb):
    x = np.asarray(x, dtype=np.float32)
    v1 = np.asarray(v1, dtype=np.float32)
    wq = np.asarray(wq, dtype=np.float32)
    wk = np.asarray(wk, dtype=np.float32)
    wv = np.asarray(wv, dtype=np.float32)
    wproj = np.asarray(wproj, dtype=np.float32)
    lam = float(np.asarray(lamb))

    cosn, sinn, tri, ztri = _host_tables()
    import ml_dtypes
    f8 = ml_dtypes.float8_e4m3

    def q3(a, scale):
        a = np.asarray(a * scale, dtype=np.float32)
        a8 = a.astype(f8)
        da8 = (a - a8.astype(np.float32)).astype(f8)
        return a8, da8

    key = lam
    if key not in _CACHE:
        _CACHE[key] = _build(lam)
    nc = _CACHE[key]

    in_maps = []
    for core in range(8):
        b, hg = core // 2, core % 2
        sl = slice(hg * HG, (hg + 1) * HG)
        x8, dx8 = q3(x[b].T, 1.0)
        wq8, dwq8 = q3(wq[sl, :].T, 64.0)
        wk8, dwk8 = q3(wk[sl, :].T, 64.0)
        wv8, dwv8 = q3(wv[sl, :].T, 64.0)
        in_maps.append({
            "x8T": np.ascontiguousarray(x8),
            "dx8T": np.ascontiguousarray(dx8),
            "v1b": np.ascontiguousarray(
                (lam * v1[b][:, sl]).astype(np.float16)),
            "wq8T": np.ascontiguousarray(wq8),
            "dwq8T": np.ascontiguousarray(dwq8),
            "wk8T": np.ascontiguousarray(wk8),
            "dwk8T": np.ascontiguousarray(dwk8),
            "wv8T": np.ascontiguousarray(wv8),
            "dwv8T": np.ascontiguousarray(dwv8),
            "wpT": np.ascontiguousarray(wproj[:, sl].T.astype(np.float16)),
            "cosn": cosn,
            "sinn": sinn,
            "tri01": tri,
            "ztri": ztri,
        })

    res = bass_utils.run_bass_kernel_spmd(nc, in_maps, core_ids=list(range(8)))
    y = np.empty((B, T, C), dtype=np.float32)
    for b in range(B):
        y[b] = (res.results[2 * b]["out"].astype(np.float32)
                + res.results[2 * b + 1]["out"].astype(np.float32))
    return (y, v1)
